# revision 1
# baseline (speedup 1.0000x reference)
"""Trainium2 Bass kernel for a 6-layer transformer decoder (B=8, S=512, D=512,
H=8, DK=DV=64, DFF=2048, vocab 32000).

Strategy: data-parallel over the batch — each of the 8 NeuronCores runs the
full decoder stack for one batch element. No collectives needed.

On-device layout: activations are kept transposed, xT[d, s], stored as SBUF
tiles [128, 4, 512] (partition = d % 128, then d-subtile, then s). Matmuls run
on the PE in MM_DT (float32r or bfloat16, fp32 PSUM accumulation). LayerNorm
statistics are computed with PE column-sum matmuls (contraction over
partitions); [1, S] rows are broadcast across partitions with K=1 matmuls and
applied with DVE divides (no slow single-lane reciprocals).

Attention per head pair: scores are computed transposed, scT[sk, sq], in four
128-row sk-chunks, interleaved across the pair so the two K=64 matmuls occupy
disjoint PE row-groups and overlap. Softmax denominators come free from an
appended all-ones column in the value projection (W_v augmented on the host
with the bias row and a ones column); a K=2 selector matmul broadcasts the
pair's two denominator rows across the 128 output partitions in one shot.
Causal masking (detected on the host) restricts matmul column ranges plus one
128x128 triangular elementwise mask per chunk; arbitrary masks fall back to
adding the (pre-scaled) mask via an identity-weight matmul into the scores
PSUM accumulation.
"""

import os
import numpy as np

_CONCOURSE_PATHS = ["/opt/trn_rl_repo", "/root/.axon_site/_ro/trn_rl_repo"]


def _ensure_path():
    try:
        import concourse.bass  # noqa: F401
    except Exception:
        import sys

        for p in _CONCOURSE_PATHS:
            if p not in sys.path and os.path.isdir(p):
                sys.path.insert(0, p)


V, D, NL, DK, DVh, H, DFF = 32000, 512, 6, 64, 64, 8, 2048
B, S = 8, 512
EPS = 1e-5
P = 128
NSUB = D // P  # 4 d-subtiles
NCH = S // P  # 4 s-chunks
NF = DFF // P  # 16 dff-chunks
HW_COLS = H * (DVh + 1)  # 520 augmented-v columns

# Debug knobs (test.py may override before calling kernel()).
N_LAYERS = NL
TAPS = ()  # e.g. ("sa0", "x1_0", "ca0", "x2_0", "ff0")
MM_DT = "f32r"  # "f32r" | "bf16"

# Results of the last kernel() call (for test.py).
LAST_RESULT = None

_BUILD_CACHE = {}


def _pe_table():
    pos = np.arange(S)[:, None].astype(np.float32)
    i = np.arange(0, D, 2).astype(np.float32)
    ang = pos / np.power(10000.0, i / D)
    pe = np.zeros((S, D), dtype=np.float32)
    pe[:, 0::2] = np.sin(ang)
    pe[:, 1::2] = np.cos(ang)
    return pe


def _to_T_tiles(mat):
    """[S, D]-like -> [P, NSUB, S] transposed-tile layout (mat.T chunked)."""
    t = np.ascontiguousarray(np.asarray(mat, np.float32)).T  # [D, S]
    return np.ascontiguousarray(t.reshape(t.shape[0] // P, P, -1).transpose(1, 0, 2))


def _col_layout(vec):
    """[D]-like -> [P, D//P] per-partition column layout."""
    v = np.asarray(vec, np.float32).reshape(-1)
    return np.ascontiguousarray(v.reshape(v.shape[0] // P, P).T)


def _build(n_layers, causal_self, self_needs_mask, cross_needs_mask, taps, mm_dt):
    _ensure_path()
    import concourse.mybir as mybir
    from concourse import bacc
    from concourse.tile import TileContext

    dt = mybir.dt
    AF = mybir.ActivationFunctionType
    OP = mybir.AluOpType
    f32 = dt.float32
    fsb = dt.float32r if mm_dt == "f32r" else dt.bfloat16
    # below 256 moving cols fp32r drops to 1/4 rate; bf16 doesn't
    n_floor = 256 if mm_dt == "f32r" else 0

    nc = bacc.Bacc("TRN2", target_bir_lowering=False, debug=False, num_devices=8)

    def din(name, shape, d=None):
        return nc.dram_tensor(name, shape, d or fsb, kind="ExternalInput")

    x0T_d = din("x0T", [P, NSUB, S])
    peT_d = din("peT", [P, NSUB, S])
    encT_d = din("encT", [P, NSUB, S])
    ones_d = din("ones_row", [1, S])
    invD_d = din("invD_col", [P, 1])
    tri_d = din("tri01", [P, P]) if causal_self else None
    ident_d = din("ident", [P, P]) if (self_needs_mask or cross_needs_mask) else None
    smask_d = din("smaskT8", [P, NCH, S]) if self_needs_mask else None
    cmask_d = din("cmaskT8", [P, NCH, S]) if cross_needs_mask else None

    wq_s_d = din("wq_s", [n_layers, P, NSUB, D])
    wk_s_d = din("wk_s", [n_layers, P, NSUB, D])
    wv_s_d = din("wv_s", [n_layers, P, NSUB, HW_COLS])
    bq_s_d = din("bq_s", [n_layers, P, NSUB], f32)
    bk_s_d = din("bk_s", [n_layers, P, NSUB], f32)
    bv_s_d = din("bv_s", [n_layers, 1, HW_COLS])
    wq_c_d = din("wq_c", [n_layers, P, NSUB, D])
    wk_c_d = din("wk_c", [n_layers, P, NSUB, D])
    wv_c_d = din("wv_c", [n_layers, P, NSUB, HW_COLS])
    bq_c_d = din("bq_c", [n_layers, P, NSUB], f32)
    bk_c_d = din("bk_c", [n_layers, P, NSUB], f32)
    bv_c_d = din("bv_c", [n_layers, 1, HW_COLS])
    w1_d = din("w1", [n_layers, P, NSUB, DFF])
    b1_d = din("b1c", [n_layers, P, NF], f32)
    w2_d = din("w2", [n_layers, P, NF, D])
    b2_d = din("b2c", [n_layers, P, NSUB], f32)
    ln1g_d = din("ln1g", [n_layers, P, NSUB], f32)
    ln1b_d = din("ln1b", [n_layers, P, NSUB], f32)
    ln2g_d = din("ln2g", [n_layers, P, NSUB], f32)
    ln2b_d = din("ln2b", [n_layers, P, NSUB], f32)

    out_d = nc.dram_tensor("out_xT", [P, NSUB, S], f32, kind="ExternalOutput")
    tap_d = {
        t: nc.dram_tensor(f"tap_{t}", [P, NSUB, S], fsb, kind="ExternalOutput")
        for t in taps
    }

    def mm(out, lhsT, rhs, start, stop):
        nc.tensor.matmul(
            out, lhsT, rhs, start=start, stop=stop, skip_group_check=True
        )

    with TileContext(nc) as tc:
        with (
            nc.allow_low_precision(reason="reduced-precision matmul pipeline"),
            tc.tile_pool(name="wts", bufs=3 if mm_dt == "f32r" else 6) as wpool,
            tc.tile_pool(name="small", bufs=14) as spool,
            tc.tile_pool(name="brows", bufs=2) as brpool,
            tc.tile_pool(name="qk", bufs=3 if mm_dt == "f32r" else 4) as qkpool,
            tc.tile_pool(name="v", bufs=2 if mm_dt == "f32r" else 3) as vpool,
            tc.tile_pool(name="exp", bufs=2 if mm_dt == "f32r" else 4) as epool,
            tc.tile_pool(name="attn", bufs=2 if mm_dt == "f32r" else 3) as apool,
            tc.tile_pool(name="x", bufs=2 if mm_dt == "f32r" else 3) as xpool,
            tc.tile_pool(name="xout", bufs=1) as xopool,
            tc.tile_pool(name="sq", bufs=1 if mm_dt == "f32r" else 2) as sqpool,
            tc.tile_pool(name="lnt", bufs=2 if mm_dt == "f32r" else 3) as tpool,
            tc.tile_pool(name="ff", bufs=1) as ffpool,
            tc.tile_pool(name="row", bufs=4 if mm_dt == "f32r" else 8) as rpool,
            tc.tile_pool(name="const", bufs=1) as cpool,
            tc.tile_pool(name="ps", bufs=7, space="PSUM") as pspool,
        ):
            # ---- constants & persistent activations ----
            ones_sb = cpool.tile([1, S], fsb, tag="c_ones")
            nc.sync.dma_start(ones_sb[:], ones_d[:])
            invD_sb = cpool.tile([P, 1], fsb, tag="c_invD")
            nc.sync.dma_start(invD_sb[:], invD_d[:])
            if causal_self:
                tri_sb = cpool.tile([P, P], fsb, tag="c_tri")
                nc.sync.dma_start(tri_sb[:], tri_d[:])
            if ident_d is not None:
                id_sb = cpool.tile([P, P], fsb, tag="c_id")
                nc.sync.dma_start(id_sb[:], ident_d[:])
            smask_sb = None
            if self_needs_mask:
                smask_sb = cpool.tile([P, NCH, S], fsb, tag="c_smask")
                nc.sync.dma_start(smask_sb[:], smask_d[:])
            cmask_sb = None
            if cross_needs_mask:
                cmask_sb = cpool.tile([P, NCH, S], fsb, tag="c_cmask")
                nc.sync.dma_start(cmask_sb[:], cmask_d[:])

            encT = cpool.tile([P, NSUB, S], fsb, tag="c_enc")
            nc.sync.dma_start(encT[:], encT_d[:])

            # x0 = emb rows (host-gathered) + positional encoding
            x0r = xpool.tile([P, NSUB, S], fsb, tag="x")
            nc.sync.dma_start(x0r[:], x0T_d[:])
            peT_sb = sqpool.tile([P, NSUB, S], fsb, tag="sq")
            nc.sync.dma_start(peT_sb[:], peT_d[:])
            xT = xpool.tile([P, NSUB, S], fsb, tag="x")
            for i in range(NSUB):
                nc.vector.tensor_tensor(
                    xT[:, i, :], x0r[:, i, :], peT_sb[:, i, :], OP.add
                )

            _psn = [0]

            def ps_tile(n=S, p=P):
                _psn[0] += 1
                return pspool.tile([p, n], f32, tag="ps", name=f"ps{_psn[0]}")

            def proj_T(w_sb, b_sb, srcT, on_act):
                """dk_all x S projection, transposed output [P, NSUB, S]."""
                t = qkpool.tile([P, NSUB, S], fsb, tag="qk")
                for j in range(NSUB):
                    ps = ps_tile()
                    for i in range(NSUB):
                        mm(
                            ps[:],
                            w_sb[:, i, j * P : (j + 1) * P],
                            srcT[:, i, :],
                            start=(i == 0),
                            stop=(i == NSUB - 1),
                        )
                    if on_act:
                        nc.scalar.activation(
                            t[:, j, :], ps[:], AF.Identity, bias=b_sb[:, j : j + 1]
                        )
                    else:
                        nc.vector.tensor_scalar(
                            t[:, j, :], ps[:], b_sb[:, j : j + 1], None, OP.add
                        )
                return t

            def v_aug(w_sb, brow_sb, srcT):
                """augmented v, natural orientation: [P(s), NCH, 520]."""
                vt = vpool.tile([P, NCH, HW_COLS], fsb, tag="v")
                half = HW_COLS // 2  # 260
                for sc in range(NCH):
                    for hh in range(2):
                        cs, ce = hh * half, (hh + 1) * half
                        ps = ps_tile(n=half)
                        for i in range(NSUB):
                            mm(
                                ps[:],
                                srcT[:, i, sc * P : (sc + 1) * P],
                                w_sb[:, i, cs:ce],
                                start=(i == 0),
                                stop=False,
                            )
                        mm(
                            ps[:],
                            ones_sb[0:1, 0:P],
                            brow_sb[0:1, cs:ce],
                            start=False,
                            stop=True,
                        )
                        nc.vector.tensor_copy(vt[:, sc, cs:ce], ps[:])
                return vt

            def attention_core(qT, kT, vt, attnT, causal, mask_sb):
                """Head pairs (2j, 2j+1): interleaved K=64 scores on disjoint
                PE row-groups, AV with fused denominator row, K=2 selector
                broadcast, DVE divide."""
                for j in range(NSUB):
                    exs = [
                        epool.tile([P, NCH, S], fsb, tag="exp", name=f"ex{j}_0"),
                        epool.tile([P, NCH, S], fsb, tag="exp", name=f"ex{j}_1"),
                    ]
                    avs = [ps_tile(), ps_tile()]
                    for c in range(NCH):
                        q0 = c * P if causal else 0
                        qs = min(q0, S - n_floor) if causal else 0
                        scs = [ps_tile(), ps_tile()]
                        for u in range(2):
                            ph = u * 64
                            mm(
                                scs[u][:, qs:S],
                                kT[ph : ph + 64, j, c * P : (c + 1) * P],
                                qT[ph : ph + 64, j, qs:S],
                                start=True,
                                stop=(mask_sb is None),
                            )
                            if mask_sb is not None:
                                mm(
                                    scs[u][:, qs:S],
                                    id_sb[:],
                                    mask_sb[:, c, qs:S],
                                    start=False,
                                    stop=True,
                                )
                        for u in range(2):
                            nc.scalar.activation(
                                exs[u][:, c, q0:S], scs[u][:, q0:S], AF.Exp,
                                scale=0.125,
                            )
                            if causal:
                                nc.vector.tensor_tensor(
                                    exs[u][:, c, c * P : (c + 1) * P],
                                    exs[u][:, c, c * P : (c + 1) * P],
                                    tri_sb[:],
                                    OP.mult,
                                )
                    for c in range(NCH):
                        q0 = c * P if causal else 0
                        for u in range(2):
                            h = 2 * j + u
                            mm(
                                avs[u][0:65, q0:S],
                                vt[:, c, h * 65 : (h + 1) * 65],
                                exs[u][:, c, q0:S],
                                start=(c == 0),
                                stop=(c == NCH - 1),
                            )
                    # per-head: broadcast the denominator row, fast
                    # approximate reciprocal (DVE has no divide), multiply
                    for u in range(2):
                        rsu = rpool.tile([1, S], fsb, tag="row", name=f"rs{j}_{u}")
                        nc.scalar.activation(rsu[:], avs[u][64:65, :], AF.Copy)
                        rb_ps = ps_tile(p=64)
                        mm(rb_ps[:], ones_sb[0:1, 0:64], rsu[0:1, :],
                           start=True, stop=True)
                        rb_sb = rpool.tile([64, S], f32, tag="row",
                                           name=f"rb{j}_{u}")
                        nc.scalar.activation(rb_sb[:], rb_ps[:], AF.Copy)
                        rcp = rpool.tile([64, S], f32, tag="row",
                                         name=f"rc{j}_{u}")
                        nc.vector.reciprocal_approx_fast(rcp[:], rb_sb[:])
                        nc.vector.tensor_tensor(
                            attnT[u * 64 : u * 64 + 64, j, :],
                            avs[u][0:64, :],
                            rcp[:],
                            OP.mult,
                        )

            def layer_norm(x_in, g_sb, b_sb, out_f32=False):
                """LN over partitions (d), per-token stats via PE sums."""
                mean_ps = ps_tile(p=1)
                s2_ps = ps_tile(p=1)
                sq = sqpool.tile([P, NSUB, S], fsb, tag="sq")
                for i in range(NSUB):
                    nc.scalar.activation(sq[:, i, :], x_in[:, i, :], AF.Square)
                for i in range(NSUB):
                    mm(
                        mean_ps[:],
                        invD_sb[:],
                        x_in[:, i, :],
                        start=(i == 0),
                        stop=(i == NSUB - 1),
                    )
                for i in range(NSUB):
                    mm(
                        s2_ps[:],
                        invD_sb[:],
                        sq[:, i, :],
                        start=(i == 0),
                        stop=(i == NSUB - 1),
                    )
                mean_sb = rpool.tile([1, S], fsb, tag="row", name="mean")
                nc.scalar.activation(mean_sb[:], mean_ps[:], AF.Copy)
                msq_sb = rpool.tile([1, S], f32, tag="row", name="msq")
                nc.vector.tensor_tensor(msq_sb[:], mean_ps[:], mean_sb[:], OP.mult)
                var_sb = rpool.tile([1, S], f32, tag="row", name="var")
                nc.vector.tensor_tensor(var_sb[:], s2_ps[:], msq_sb[:], OP.subtract)
                nc.vector.tensor_scalar(var_sb[:], var_sb[:], float(EPS), None, OP.add)
                sd_sb = rpool.tile([1, S], f32, tag="row", name="sd")
                nc.scalar.activation(sd_sb[:], var_sb[:], AF.Sqrt)
                rsd_f = rpool.tile([1, S], f32, tag="row", name="rsdf")
                nc.vector.reciprocal_approx_fast(rsd_f[:], sd_sb[:])
                rsd = rpool.tile([1, S], fsb, tag="row", name="rsd")
                nc.scalar.activation(rsd[:], rsd_f[:], AF.Copy)
                mb_ps = ps_tile()
                mm(mb_ps[:], ones_sb[0:1, 0:P], mean_sb[0:1, :], start=True, stop=True)
                sdb_ps = ps_tile()
                mm(sdb_ps[:], ones_sb[0:1, 0:P], rsd[0:1, :], start=True, stop=True)
                xo = (xopool if out_f32 else xpool).tile(
                    [P, NSUB, S],
                    f32 if out_f32 else fsb,
                    tag="xo" if out_f32 else "x",
                )
                for i in range(NSUB):
                    t1 = tpool.tile([P, S], f32, tag="lnt")
                    nc.vector.tensor_tensor(t1[:], x_in[:, i, :], mb_ps[:], OP.subtract)
                    nc.vector.tensor_tensor(t1[:], t1[:], sdb_ps[:], OP.mult)
                    nc.scalar.activation(
                        xo[:, i, :],
                        t1[:],
                        AF.Identity,
                        bias=b_sb[:, i : i + 1],
                        scale=g_sb[:, i : i + 1],
                    )
                return xo

            def residual(a_T, b_T):
                xo = xpool.tile([P, NSUB, S], fsb, tag="x")
                for i in range(NSUB):
                    nc.vector.tensor_tensor(
                        xo[:, i, :], a_T[:, i, :], b_T[:, i, :], OP.add
                    )
                return xo

            def load_w(src, l, shape):
                t = wpool.tile(shape, fsb, tag="wt")
                nc.sync.dma_start(t[:], src[l])
                return t

            def load_small(src, l, shape, tag):
                if tag == "brow":
                    t = brpool.tile(shape, fsb, tag=tag)
                else:
                    t = spool.tile(shape, f32, tag=tag)
                nc.sync.dma_start(t[:], src[l])
                return t

            def tap(name, tile_):
                if name in tap_d:
                    nc.sync.dma_start(tap_d[name][:], tile_[:])

            for l in range(n_layers):
                # ---- self attention ----
                wq = load_w(wq_s_d, l, [P, NSUB, D])
                wk = load_w(wk_s_d, l, [P, NSUB, D])
                wv = load_w(wv_s_d, l, [P, NSUB, HW_COLS])
                bq = load_small(bq_s_d, l, [P, NSUB], "bcol")
                bk = load_small(bk_s_d, l, [P, NSUB], "bcol")
                bv = load_small(bv_s_d, l, [1, HW_COLS], "brow")
                qT = proj_T(wq, bq, xT, on_act=True)
                kT = proj_T(wk, bk, xT, on_act=False)
                vt = v_aug(wv, bv, xT)
                saT = apool.tile([P, NSUB, S], fsb, tag="attn")
                attention_core(qT, kT, vt, saT, causal_self, smask_sb)
                tap(f"sa{l}", saT)

                # cross K/V from the encoder — independent of LN1, emitted here
                # so the PE has work while LN1's vector chain runs
                wkc = load_w(wk_c_d, l, [P, NSUB, D])
                wvc = load_w(wv_c_d, l, [P, NSUB, HW_COLS])
                bkc = load_small(bk_c_d, l, [P, NSUB], "bcol")
                bvc = load_small(bv_c_d, l, [1, HW_COLS], "brow")
                kcT = proj_T(wkc, bkc, encT, on_act=False)
                vc = v_aug(wvc, bvc, encT)

                g1 = load_small(ln1g_d, l, [P, NSUB], "bcol")
                b1c_ln = load_small(ln1b_d, l, [P, NSUB], "bcol")
                x1 = layer_norm(residual(xT, saT), g1, b1c_ln)
                tap(f"x1_{l}", x1)

                # ---- cross attention ----
                wqc = load_w(wq_c_d, l, [P, NSUB, D])
                bqc = load_small(bq_c_d, l, [P, NSUB], "bcol")
                qcT = proj_T(wqc, bqc, x1, on_act=True)
                caT = apool.tile([P, NSUB, S], fsb, tag="attn")
                attention_core(qcT, kcT, vc, caT, False, cmask_sb)
                tap(f"ca{l}", caT)
                g2 = load_small(ln2g_d, l, [P, NSUB], "bcol")
                b2c_ln = load_small(ln2b_d, l, [P, NSUB], "bcol")
                x2 = layer_norm(residual(x1, caT), g2, b2c_ln)
                tap(f"x2_{l}", x2)

                # ---- FFN ----
                b1col = load_small(b1_d, l, [P, NF], "b1col")
                ff1 = ffpool.tile([P, NF, S], fsb, tag="ff1")
                for g in range(4):  # w1 granules of 512 dff cols
                    w1g = wpool.tile([P, NSUB, 512], fsb, tag="wt")
                    nc.sync.dma_start(
                        w1g[:], w1_d[l, :, :, g * 512 : (g + 1) * 512]
                    )
                    for fl in range(4):
                        F = g * 4 + fl
                        ps = ps_tile()
                        for i in range(NSUB):
                            mm(
                                ps[:],
                                w1g[:, i, fl * P : (fl + 1) * P],
                                x2[:, i, :],
                                start=(i == 0),
                                stop=(i == NSUB - 1),
                            )
                        nc.scalar.activation(
                            ff1[:, F, :], ps[:], AF.Relu, bias=b1col[:, F : F + 1]
                        )
                b2col = load_small(b2_d, l, [P, NSUB], "bcol")
                ffo = apool.tile([P, NSUB, S], fsb, tag="attn")
                for j in range(NSUB):
                    w2g = wpool.tile([P, NF, P], fsb, tag="wt")
                    nc.sync.dma_start(w2g[:], w2_d[l, :, :, j * P : (j + 1) * P])
                    ps = ps_tile()
                    for k in range(NF):
                        mm(
                            ps[:],
                            w2g[:, k, :],
                            ff1[:, k, :],
                            start=(k == 0),
                            stop=(k == NF - 1),
                        )
                    nc.scalar.activation(
                        ffo[:, j, :], ps[:], AF.Identity, bias=b2col[:, j : j + 1]
                    )
                tap(f"ff{l}", ffo)
                xT = layer_norm(
                    residual(x2, ffo), g2, b2c_ln, out_f32=(l == n_layers - 1)
                )

            nc.sync.dma_start(out_d[:], xT[:])

    nc.compile()
    return nc


def _prep_shared(inputs, n_layers):
    """Host-side marshalling of weights into device tile layouts (float32;
    kernel() casts matmul-side arrays to the MM_DT numpy dtype)."""
    g = {}
    emb = np.asarray(inputs["emb"], np.float32)

    def wqk_prep(w):  # [NL, H, D, DK] -> [nl, P, NSUB, D]
        out = np.empty((n_layers, P, NSUB, D), np.float32)
        for l in range(n_layers):
            w2d = np.asarray(w[l], np.float32).transpose(1, 0, 2).reshape(D, H * DK)
            out[l] = w2d.reshape(NSUB, P, H * DK).transpose(1, 0, 2)
        return np.ascontiguousarray(out)

    def wv_prep(w, bv):  # augmented: per head 64 v-cols + ones col
        wout = np.empty((n_layers, P, NSUB, HW_COLS), np.float32)
        brow = np.zeros((n_layers, 1, HW_COLS), np.float32)
        for l in range(n_layers):
            aug = np.zeros((D, HW_COLS), np.float32)
            baug = np.zeros(HW_COLS, np.float32)
            wl = np.asarray(w[l], np.float32)  # [H, D, DVh]
            bl = np.asarray(bv[l], np.float32)  # [H, DVh]
            for h in range(H):
                aug[:, h * 65 : h * 65 + 64] = wl[h]
                baug[h * 65 : h * 65 + 64] = bl[h]
                baug[h * 65 + 64] = 1.0
            wout[l] = aug.reshape(NSUB, P, HW_COLS).transpose(1, 0, 2)
            brow[l, 0] = baug
        return np.ascontiguousarray(wout), brow

    def bcol_prep(b):  # [NL, ...] -> [nl, P, width]
        out = np.stack(
            [_col_layout(np.asarray(b[l], np.float32)) for l in range(n_layers)]
        )
        return np.ascontiguousarray(out)

    g["wq_s"] = wqk_prep(inputs["Wq_s"])
    g["wk_s"] = wqk_prep(inputs["Wk_s"])
    g["wv_s"], g["bv_s"] = wv_prep(inputs["Wv_s"], inputs["bv_s"])
    g["bq_s"] = bcol_prep(inputs["bq_s"])
    g["bk_s"] = bcol_prep(inputs["bk_s"])
    g["wq_c"] = wqk_prep(inputs["Wq_c"])
    g["wk_c"] = wqk_prep(inputs["Wk_c"])
    g["wv_c"], g["bv_c"] = wv_prep(inputs["Wv_c"], inputs["bv_c"])
    g["bq_c"] = bcol_prep(inputs["bq_c"])
    g["bk_c"] = bcol_prep(inputs["bk_c"])

    w1 = np.empty((n_layers, P, NSUB, DFF), np.float32)
    w2 = np.empty((n_layers, P, NF, D), np.float32)
    for l in range(n_layers):
        w1[l] = (
            np.asarray(inputs["W1"][l], np.float32)
            .reshape(NSUB, P, DFF)
            .transpose(1, 0, 2)
        )
        w2[l] = (
            np.asarray(inputs["W2"][l], np.float32)
            .reshape(NF, P, D)
            .transpose(1, 0, 2)
        )
    g["w1"] = np.ascontiguousarray(w1)
    g["w2"] = np.ascontiguousarray(w2)
    g["b1c"] = bcol_prep(inputs["b1"])
    g["b2c"] = bcol_prep(inputs["b2"])
    g["ln1g"] = bcol_prep(inputs["ln1_g"])
    g["ln1b"] = bcol_prep(inputs["ln1_b"])
    g["ln2g"] = bcol_prep(inputs["ln2_g"])
    g["ln2b"] = bcol_prep(inputs["ln2_b"])

    g["peT"] = _to_T_tiles(_pe_table())
    g["ones_row"] = np.ones((1, S), np.float32)
    g["invD_col"] = np.full((P, 1), 1.0 / D, np.float32)
    sel2 = np.zeros((2, P), np.float32)
    sel2[0, 0:64] = 1.0
    sel2[1, 64:128] = 1.0
    g["sel2"] = sel2
    q = np.arange(P)
    g["tri01"] = (q[None, :] >= q[:, None]).astype(np.float32)
    g["ident"] = np.eye(P, dtype=np.float32)
    return g, emb


def _mask_T8(mask_b):
    """[S, S] additive mask -> [P, NCH, S] transposed, pre-scaled by 8."""
    m = np.ascontiguousarray(np.asarray(mask_b, np.float32).T) * 8.0
    return np.ascontiguousarray(m.reshape(NCH, P, S).transpose(1, 0, 2))


# f32 bias-column tensors; everything else carries the matmul dtype
_F32_KEYS = {
    "bq_s", "bk_s", "bq_c", "bk_c", "b1c", "b2c",
    "ln1g", "ln1b", "ln2g", "ln2b",
}


def kernel(**inputs):
    global LAST_RESULT
    _ensure_path()
    import ml_dtypes
    from concourse.bass_utils import run_bass_kernel_spmd

    n_layers = N_LAYERS
    mm_np = np.float32 if MM_DT == "f32r" else ml_dtypes.bfloat16
    ids = np.asarray(inputs["decoder_input"])
    enc = np.asarray(inputs["encoder_output"], np.float32)
    smask = np.asarray(inputs["self_mask"], np.float32)
    cmask = np.asarray(inputs["cross_mask"], np.float32)

    tril = np.tril(np.ones((S, S), bool))
    canon = np.where(tril, np.float32(0.0), np.float32(-1e9))
    causal_self = all(np.array_equal(smask[b], canon) for b in range(B))
    self_needs_mask = (not causal_self) and bool(np.any(smask != 0.0))
    cross_needs_mask = bool(np.any(cmask != 0.0))

    shared, emb = _prep_shared(inputs, n_layers)
    shared.pop("sel2", None)
    shared = {
        k: (v if k in _F32_KEYS else v.astype(mm_np)) for k, v in shared.items()
    }

    key = (n_layers, causal_self, self_needs_mask, cross_needs_mask, tuple(TAPS), MM_DT)
    if key not in _BUILD_CACHE:
        _BUILD_CACHE[key] = _build(
            n_layers, causal_self, self_needs_mask, cross_needs_mask, tuple(TAPS),
            MM_DT,
        )
    nc = _BUILD_CACHE[key]

    in_maps = []
    for b in range(B):
        m = dict(shared)
        m["x0T"] = _to_T_tiles(emb[ids[b]]).astype(mm_np)
        m["encT"] = _to_T_tiles(enc[b]).astype(mm_np)
        if self_needs_mask:
            m["smaskT8"] = _mask_T8(smask[b]).astype(mm_np)
        if cross_needs_mask:
            m["cmaskT8"] = _mask_T8(cmask[b]).astype(mm_np)
        if not causal_self:
            m.pop("tri01", None)
        if not (self_needs_mask or cross_needs_mask):
            m.pop("ident", None)
        in_maps.append(m)

    res = run_bass_kernel_spmd(nc, in_maps, core_ids=list(range(8)))
    LAST_RESULT = res

    out = np.empty((B, S, D), np.float32)
    for b in range(B):
        xt = np.asarray(res.results[b]["out_xT"], np.float32)  # [P, NSUB, S]
        out[b] = xt.transpose(1, 0, 2).reshape(D, S).T
    return out



# revision 18
# speedup vs baseline: 1.0852x; 1.0852x over previous
"""Trainium2 Bass kernel for a 6-layer transformer decoder (B=8, S=512, D=512,
H=8, DK=DV=64, DFF=2048, vocab 32000).

Strategy: data-parallel over the batch - each of the 8 NeuronCores runs the
full decoder stack for one batch element. No collectives.

v2 (HAM-aware restructure): everything matmul-side runs in bf16 (fp32 PSUM
accumulation). The PE instruction stream is kept dense so the HAM clock gate
stays at 2.4 GHz:
  - attention is software-pipelined across head pairs (scores_{j+1} is queued
    on the PE before head-pair j's denominator broadcast, so the PE never
    waits on the DVE/ACT softmax chain);
  - the encoder-side cross-attention K/V projections are floated into the
    LayerNorm row-chain bubbles (they depend only on the encoder output);
  - LayerNorm stats matmuls are emitted per-subtile as residuals complete,
    and the first projection after each LN accumulates input-subtile-outer
    so it starts as soon as apply writes subtile 0.
Engine balance: exp/softmax + PSUM->SBUF casts on ACT (one activation table:
exp/ln/copy/identity/square - 1/sigma is exp(-0.5*ln(var+eps)), no table
reloads), elementwise + reciprocal + relu on DVE, residuals/squares/causal
masking/half the LN applies on GpSimd (Pool).
"""

import os
import numpy as np

_CONCOURSE_PATHS = ["/opt/trn_rl_repo", "/root/.axon_site/_ro/trn_rl_repo"]


def _ensure_path():
    try:
        import concourse.bass  # noqa: F401
    except Exception:
        import sys

        for p in _CONCOURSE_PATHS:
            if p not in sys.path and os.path.isdir(p):
                sys.path.insert(0, p)


V, D, NL, DK, DVh, H, DFF = 32000, 512, 6, 64, 64, 8, 2048
B, S = 8, 512
EPS = 1e-5
P = 128
NSUB = D // P  # 4 d-subtiles
NCH = S // P  # 4 s-chunks
NF = DFF // P  # 16 dff-chunks
HW_COLS = H * (DVh + 1)  # 520 augmented-v columns

# Debug knobs (test.py may override before calling kernel()).
N_LAYERS = NL
TAPS = ()  # e.g. ("sa0", "x1_0", "ca0", "x2_0", "ff0")
MM_DT = "bf16"  # kept for test.py compat; build is bf16-only

# Results of the last kernel() call (for test.py).
LAST_RESULT = None

_BUILD_CACHE = {}


def _pe_table():
    pos = np.arange(S)[:, None].astype(np.float32)
    i = np.arange(0, D, 2).astype(np.float32)
    ang = pos / np.power(10000.0, i / D)
    pe = np.zeros((S, D), dtype=np.float32)
    pe[:, 0::2] = np.sin(ang)
    pe[:, 1::2] = np.cos(ang)
    return pe


def _to_T_tiles(mat):
    """[S, D]-like -> [P, NSUB, S] transposed-tile layout (mat.T chunked)."""
    t = np.ascontiguousarray(np.asarray(mat, np.float32)).T  # [D, S]
    return np.ascontiguousarray(t.reshape(t.shape[0] // P, P, -1).transpose(1, 0, 2))


def _col_layout(vec):
    """[D]-like -> [P, D//P] per-partition column layout."""
    v = np.asarray(vec, np.float32).reshape(-1)
    return np.ascontiguousarray(v.reshape(v.shape[0] // P, P).T)


def _build(n_layers, causal_self, self_needs_mask, cross_needs_mask, lean_ln, taps):
    _ensure_path()
    import concourse.mybir as mybir
    from concourse import bacc
    from concourse.tile import TileContext

    dt = mybir.dt
    AF = mybir.ActivationFunctionType
    OP = mybir.AluOpType
    f32 = dt.float32
    fsb = dt.bfloat16

    nc = bacc.Bacc("TRN2", target_bir_lowering=False, debug=False, num_devices=8)

    def din(name, shape, d=None):
        return nc.dram_tensor(name, shape, d or fsb, kind="ExternalInput")

    x0T_d = din("x0T", [P, NSUB, S])  # emb rows + positional enc (host)
    encT_d = din("encT", [P, NSUB, S])
    ones_d = din("ones_row", [1, S])
    invD_d = din("invD_col", [P, 1])
    sel2_d = din("sel2", [2, P])
    tri_d = din("tri01", [P, P]) if causal_self else None
    ident_d = din("ident", [P, P]) if (self_needs_mask or cross_needs_mask) else None
    smask_d = din("smaskT8", [P, NCH, S]) if self_needs_mask else None
    cmask_d = din("cmaskT8", [P, NCH, S]) if cross_needs_mask else None

    wq_s_d = din("wq_s", [n_layers, P, NSUB, D])
    wk_s_d = din("wk_s", [n_layers, P, NSUB, D])
    wv_s_d = din("wv_s", [n_layers, P, NSUB, HW_COLS])
    bq_s_d = din("bq_s", [n_layers, P, NSUB], f32)
    bk_s_d = din("bk_s", [n_layers, P, NSUB], f32)
    bv_s_d = din("bv_s", [n_layers, 1, HW_COLS])
    wq_c_d = din("wq_c", [n_layers, P, NSUB, D])
    wk_c_d = din("wk_c", [n_layers, P, NSUB, D])
    wv_c_d = din("wv_c", [n_layers, P, NSUB, HW_COLS])
    bq_c_d = din("bq_c", [n_layers, P, NSUB], f32)
    bk_c_d = din("bk_c", [n_layers, P, NSUB], f32)
    bv_c_d = din("bv_c", [n_layers, 1, HW_COLS])
    w1_d = din("w1", [n_layers, P, NSUB, DFF])
    b1_d = din("b1c", [n_layers, P, NF], f32)
    w2_d = din("w2", [n_layers, P, NF, D])
    b2_d = din("b2c", [n_layers, P, NSUB], f32)
    if not lean_ln:
        ln1g_d = din("ln1g", [n_layers, P, NSUB], f32)
        ln1b_d = din("ln1b", [n_layers, P, NSUB], f32)
        ln2g_d = din("ln2g", [n_layers, P, NSUB], f32)
        ln2b_d = din("ln2b", [n_layers, P, NSUB], f32)

    out_d = nc.dram_tensor("out_xT", [P, NSUB, S], f32, kind="ExternalOutput")
    tap_d = {
        t: nc.dram_tensor(f"tap_{t}", [P, NSUB, S], fsb, kind="ExternalOutput")
        for t in taps
    }

    def mm(out, lhsT, rhs, start, stop):
        nc.tensor.matmul(
            out, lhsT, rhs, start=start, stop=stop, skip_group_check=True
        )

    from contextlib import ExitStack

    with TileContext(nc) as tc:
        with ExitStack() as stack:
            en = stack.enter_context
            en(nc.allow_low_precision(reason="bf16 matmul pipeline"))
            wbpool = en(tc.tile_pool(name="wbig", bufs=2))
            wpool = en(tc.tile_pool(name="wsm", bufs=7))
            spool = en(tc.tile_pool(name="small", bufs=12))
            brpool = en(tc.tile_pool(name="brows", bufs=4))
            qkpool = en(tc.tile_pool(name="qk", bufs=2))
            kcpool = en(tc.tile_pool(name="kc", bufs=2))
            vpool = en(tc.tile_pool(name="v", bufs=2))
            epool = en(tc.tile_pool(name="exp", bufs=4))
            apool = en(tc.tile_pool(name="attn", bufs=2))
            xpool = en(tc.tile_pool(name="x", bufs=2))
            xipool = en(tc.tile_pool(name="xin", bufs=2))
            sqpool = en(tc.tile_pool(name="sq", bufs=2))
            xopool = en(tc.tile_pool(name="xout", bufs=1))
            tpool = en(tc.tile_pool(name="t1", bufs=3))
            rbpool = en(tc.tile_pool(name="rb", bufs=2))
            mbpool = en(tc.tile_pool(name="mbsb", bufs=4))
            ffpool = en(tc.tile_pool(name="ff", bufs=1))
            rpool = en(tc.tile_pool(name="row", bufs=4))
            rcpool = en(tc.tile_pool(name="rcp", bufs=3))
            cpool = en(tc.tile_pool(name="const", bufs=1))
            pspool = en(tc.tile_pool(name="ps", bufs=8, space="PSUM"))
            # ---- constants & persistent activations ----
            ones_sb = cpool.tile([1, S], fsb, tag="c_ones")
            nc.sync.dma_start(ones_sb[:], ones_d[:])
            invD_sb = cpool.tile([P, 1], fsb, tag="c_invD")
            nc.sync.dma_start(invD_sb[:], invD_d[:])
            sel_sb = []
            for u in range(2):
                t = cpool.tile([1, P], fsb, tag=f"c_sel{u}", name=f"sel{u}")
                nc.sync.dma_start(t[:], sel2_d[u : u + 1, :])
                sel_sb.append(t)
            if causal_self:
                tri_sb = cpool.tile([P, P], fsb, tag="c_tri")
                nc.sync.dma_start(tri_sb[:], tri_d[:])
            id_sb = None
            if ident_d is not None:
                id_sb = cpool.tile([P, P], fsb, tag="c_id")
                nc.sync.dma_start(id_sb[:], ident_d[:])
            smask_sb = None
            if self_needs_mask:
                smask_sb = cpool.tile([P, NCH, S], fsb, tag="c_smask")
                nc.sync.dma_start(smask_sb[:], smask_d[:])
            cmask_sb = None
            if cross_needs_mask:
                cmask_sb = cpool.tile([P, NCH, S], fsb, tag="c_cmask")
                nc.sync.dma_start(cmask_sb[:], cmask_d[:])

            eps_sb = cpool.tile([1, 1], f32, tag="c_eps")
            nc.gpsimd.memset(eps_sb[:], float(EPS))

            encT = cpool.tile([P, NSUB, S], fsb, tag="c_enc")
            nc.sync.dma_start(encT[:], encT_d[:])

            xT = xpool.tile([P, NSUB, S], fsb, tag="x")
            nc.sync.dma_start(xT[:], x0T_d[:])

            _psn = [0]

            def ps_tile(n=S, p=P):
                _psn[0] += 1
                return pspool.tile([p, n], f32, tag="ps", name=f"ps{_psn[0]}")

            def load_w(src, l, shape, pool=None):
                t = (pool or wpool).tile(shape, fsb, tag="wt")
                nc.sync.dma_start(t[:], src[l])
                return t

            def load_small(src, l, shape, tag="bcol"):
                if tag == "brow":
                    t = brpool.tile(shape, fsb, tag=tag)
                else:
                    t = spool.tile(shape, f32, tag=tag)
                nc.sync.dma_start(t[:], src[l])
                return t

            def tap(name, tile_):
                if name in tap_d:
                    nc.sync.dma_start(tap_d[name][:], tile_[:])

            def proj_T(w_sb, b_sb, srcT, pool, ptag, i_outer, on_act):
                """dk_all x S projection, transposed output [P, NSUB, S].

                i_outer: accumulate input-subtile-outer so the first matmul
                issues as soon as srcT[:, 0, :] is written."""
                t = pool.tile([P, NSUB, S], fsb, tag=ptag)
                if i_outer:
                    pss = [ps_tile() for _ in range(NSUB)]
                    for i in range(NSUB):
                        for j in range(NSUB):
                            mm(
                                pss[j][:],
                                w_sb[:, i, j * P : (j + 1) * P],
                                srcT[:, i, :],
                                start=(i == 0),
                                stop=(i == NSUB - 1),
                            )
                    for j in range(NSUB):
                        if on_act:
                            nc.scalar.activation(
                                t[:, j, :], pss[j][:], AF.Identity,
                                bias=b_sb[:, j : j + 1],
                            )
                        else:
                            nc.vector.tensor_scalar(
                                t[:, j, :], pss[j][:], b_sb[:, j : j + 1], None,
                                OP.add,
                            )
                else:
                    for j in range(NSUB):
                        ps = ps_tile()
                        for i in range(NSUB):
                            mm(
                                ps[:],
                                w_sb[:, i, j * P : (j + 1) * P],
                                srcT[:, i, :],
                                start=(i == 0),
                                stop=(i == NSUB - 1),
                            )
                        if on_act:
                            nc.scalar.activation(
                                t[:, j, :], ps[:], AF.Identity,
                                bias=b_sb[:, j : j + 1],
                            )
                        else:
                            nc.vector.tensor_scalar(
                                t[:, j, :], ps[:], b_sb[:, j : j + 1], None, OP.add
                            )
                return t

            def v_aug_part(vt, w_sb, brow_sb, srcT, sc_list):
                """augmented v, natural orientation: [P(s), NCH, 520]."""
                half = HW_COLS // 2  # 260
                for sc in sc_list:
                    for hh in range(2):
                        cs, ce = hh * half, (hh + 1) * half
                        ps = ps_tile(n=half)
                        for i in range(NSUB):
                            mm(
                                ps[:],
                                srcT[:, i, sc * P : (sc + 1) * P],
                                w_sb[:, i, cs:ce],
                                start=(i == 0),
                                stop=False,
                            )
                        mm(
                            ps[:],
                            ones_sb[0:1, 0:P],
                            brow_sb[0:1, cs:ce],
                            start=False,
                            stop=True,
                        )
                        nc.vector.tensor_copy(vt[:, sc, cs:ce], ps[:])

            def attention(qT, kT, vt, outT, causal, mask_sb, post_subtile):
                """Software-pipelined attention. Head pair j = heads (2j,2j+1).

                Scores are computed transposed scT[sk, sq] in 128-row sk
                chunks, the two heads of a pair on disjoint PE row groups.
                Softmax denominators come from the augmented-v ones column
                (row 64 of the AV accumulation); 1/denom on DVE, broadcast
                across the pair's 128 output partitions with one K=2 selector
                matmul, one ACT cast to SBUF, multiply on DVE.
                post_subtile(j) emits the follow-on work for subtile j."""
                nj = NSUB
                st = [dict() for _ in range(nj)]

                def s_stage(j):
                    d = st[j]
                    d["exs"] = [
                        epool.tile([P, NCH, S], fsb, tag="exp", name=f"ex{j}_0"),
                        epool.tile([P, NCH, S], fsb, tag="exp", name=f"ex{j}_1"),
                    ]
                    for c in range(NCH):
                        q0 = c * P if causal else 0
                        scs = [ps_tile(), ps_tile()]
                        for u in range(2):
                            ph = u * 64
                            mm(
                                scs[u][:, q0:S],
                                kT[ph : ph + 64, j, c * P : (c + 1) * P],
                                qT[ph : ph + 64, j, q0:S],
                                start=True,
                                stop=(mask_sb is None),
                            )
                            if mask_sb is not None:
                                mm(
                                    scs[u][:, q0:S],
                                    id_sb[:],
                                    mask_sb[:, c, q0:S],
                                    start=False,
                                    stop=True,
                                )
                        for u in range(2):
                            nc.scalar.activation(
                                d["exs"][u][:, c, q0:S], scs[u][:, q0:S], AF.Exp,
                                scale=0.125,
                            )
                            if causal:
                                nc.gpsimd.tensor_tensor(
                                    d["exs"][u][:, c, c * P : (c + 1) * P],
                                    d["exs"][u][:, c, c * P : (c + 1) * P],
                                    tri_sb[:],
                                    OP.mult,
                                )

                def av_stage(j):
                    d = st[j]
                    d["avs"] = [ps_tile(p=65), ps_tile(p=65)]
                    for c in range(NCH):
                        q0 = c * P if causal else 0
                        for u in range(2):
                            h = 2 * j + u
                            mm(
                                d["avs"][u][0:65, q0:S],
                                vt[:, c, h * 65 : (h + 1) * 65],
                                d["exs"][u][:, c, q0:S],
                                start=(c == 0),
                                stop=(c == NCH - 1),
                            )

                def rcpb_stage(j):
                    # denominator rows (bf16, partition 0) -> two K=1 selector
                    # matmuls accumulate the pair broadcast -> one DVE
                    # reciprocal does the PSUM->SBUF move
                    d = st[j]
                    rb2 = ps_tile()
                    for u in range(2):
                        du = rcpool.tile([1, S], fsb, tag="rcp",
                                         name=f"d{j}_{u}")
                        nc.scalar.activation(
                            du[:], d["avs"][u][64:65, :], AF.Copy
                        )
                        mm(
                            rb2[:], sel_sb[u][0:1, :], du[0:1, :],
                            start=(u == 0), stop=(u == 1),
                        )
                    d["rb2sb"] = rbpool.tile([P, S], f32, tag="rb", name=f"rb{j}")
                    nc.vector.reciprocal_approx_fast(d["rb2sb"][:], rb2[:])

                def mult_stage(j):
                    d = st[j]
                    for u in range(2):
                        nc.vector.tensor_tensor(
                            outT[u * 64 : u * 64 + 64, j, :],
                            d["avs"][u][0:64, :],
                            d["rb2sb"][u * 64 : u * 64 + 64, :],
                            OP.mult,
                        )
                    post_subtile(j)

                # emission: PE queue = s0 s1 av0 s2 b0 av1 s3 b1 av2 b2 av3 b3
                s_stage(0)
                s_stage(1)
                av_stage(0)
                for j in range(2, nj):
                    s_stage(j)
                    rcpb_stage(j - 2)
                    av_stage(j - 1)
                    mult_stage(j - 2)
                rcpb_stage(nj - 2)
                av_stage(nj - 1)
                mult_stage(nj - 2)
                rcpb_stage(nj - 1)
                mult_stage(nj - 1)

            def mk_post(base_T, add_T, x_in, sq):
                """residual + square on Pool, per subtile."""

                def post(j):
                    nc.gpsimd.tensor_tensor(
                        x_in[:, j, :], base_T[:, j, :], add_T[:, j, :], OP.add
                    )
                    nc.gpsimd.tensor_tensor(
                        sq[:, j, :], x_in[:, j, :], x_in[:, j, :], OP.mult
                    )

                return post

            def ln_stats(x_in, sq):
                mean_ps, s2_ps = ps_tile(p=1), ps_tile(p=1)
                for j in range(NSUB):
                    mm(
                        mean_ps[:],
                        invD_sb[:],
                        x_in[:, j, :],
                        start=(j == 0),
                        stop=(j == NSUB - 1),
                    )
                for j in range(NSUB):
                    mm(
                        s2_ps[:],
                        invD_sb[:],
                        sq[:, j, :],
                        start=(j == 0),
                        stop=(j == NSUB - 1),
                    )
                return mean_ps, s2_ps

            def ln_finish(x_in, mean_ps, s2_ps, g_sb, b_sb, out_pool, out_tag,
                          out_f32=False):
                """Row chain + broadcast + apply (DVE/Pool split)."""
                mean_sb = rpool.tile([1, S], fsb, tag="row", name="mean")
                nc.scalar.activation(mean_sb[:], mean_ps[:], AF.Copy)
                msq = rpool.tile([1, S], f32, tag="row", name="msq")
                nc.scalar.activation(msq[:], mean_ps[:], AF.Square)
                var = rpool.tile([1, S], f32, tag="row", name="var")
                nc.vector.tensor_tensor(var[:], s2_ps[:], msq[:], OP.subtract)
                lnv = rpool.tile([1, S], f32, tag="row", name="lnv")
                nc.scalar.activation(lnv[:], var[:], AF.Ln, bias=eps_sb[0:1, 0:1])
                rsd = rpool.tile([1, S], fsb, tag="row", name="rsd")
                nc.scalar.activation(rsd[:], lnv[:], AF.Exp, scale=-0.5)
                mb_ps = ps_tile()
                mm(mb_ps[:], ones_sb[0:1, 0:P], mean_sb[0:1, :], start=True,
                   stop=True)
                sdb_ps = ps_tile()
                mm(sdb_ps[:], ones_sb[0:1, 0:P], rsd[0:1, :], start=True, stop=True)
                mb_sb = mbpool.tile([P, S], fsb, tag="mb", name="mb")
                nc.scalar.activation(mb_sb[:], mb_ps[:], AF.Copy)
                sdb_sb = mbpool.tile([P, S], fsb, tag="mb", name="sdb")
                nc.scalar.activation(sdb_sb[:], sdb_ps[:], AF.Copy)
                xo = out_pool.tile(
                    [P, NSUB, S], f32 if out_f32 else fsb, tag=out_tag
                )
                for i in range(NSUB):
                    eng = nc.vector if i % 2 == 0 else nc.gpsimd
                    t1 = tpool.tile([P, S], fsb, tag="t1", name=f"t1_{i}")
                    eng.tensor_tensor(t1[:], x_in[:, i, :], mb_sb[:], OP.subtract)
                    if lean_ln:
                        eng.tensor_tensor(
                            xo[:, i, :], t1[:], sdb_sb[:], OP.mult
                        )
                    else:
                        t2 = tpool.tile([P, S], fsb, tag="t1", name=f"t2_{i}")
                        eng.tensor_tensor(t2[:], t1[:], sdb_sb[:], OP.mult)
                        eng.tensor_scalar(
                            xo[:, i, :], t2[:], g_sb[:, i : i + 1],
                            b_sb[:, i : i + 1], OP.mult, OP.add,
                        )
                return xo

            # ---- weight prefetch state ----
            kc_w = {}  # l -> (wkc tile, bkc)
            vc_w = {}  # l -> (wvc tile, bvc)
            sa_w = {}  # l -> (wq, wk, wv, bq, bk, bv)
            kc_state = {}  # l -> kcT tile
            vc_state = {}  # l -> [vt tile, w_sb, brow_sb, remaining sc list]

            def load_sa(l):
                sa_w[l] = (
                    load_w(wq_s_d, l, [P, NSUB, D]),
                    load_w(wk_s_d, l, [P, NSUB, D]),
                    load_w(wv_s_d, l, [P, NSUB, HW_COLS]),
                    load_small(bq_s_d, l, [P, NSUB]),
                    load_small(bk_s_d, l, [P, NSUB]),
                    load_small(bv_s_d, l, [1, HW_COLS], "brow"),
                )

            def load_kc(l):
                kc_w[l] = (
                    load_w(wk_c_d, l, [P, NSUB, D]),
                    load_small(bk_c_d, l, [P, NSUB]),
                )

            def load_vc(l):
                vc_w[l] = (
                    load_w(wv_c_d, l, [P, NSUB, HW_COLS]),
                    load_small(bv_c_d, l, [1, HW_COLS], "brow"),
                )

            def emit_kc(l):
                wkc, bkc = kc_w.pop(l)
                kc_state[l] = proj_T(
                    wkc, bkc, encT, kcpool, "kc", i_outer=False, on_act=False
                )

            def start_vc(l):
                wvc, bvc = vc_w.pop(l)
                vt = vpool.tile([P, NCH, HW_COLS], fsb, tag="v", name=f"vc{l}")
                vc_state[l] = [vt, wvc, bvc, list(range(NCH))]

            def emit_vc_part(l, k=None):
                vt, wvc, bvc, rem = vc_state[l]
                k = len(rem) if k is None else k
                scs, vc_state[l][3] = rem[:k], rem[k:]
                v_aug_part(vt, wvc, bvc, encT, scs)

            load_sa(0)
            load_kc(0)
            load_vc(0)
            for l in range(n_layers):
                # ---- self attention ----
                wq, wk, wv, bq, bk, bv = sa_w.pop(l)
                # big FFN weights + next layer's cross weights: DMAs start now
                w1sb = load_w(w1_d, l, [P, NSUB, DFF], wbpool)
                w2sb = load_w(w2_d, l, [P, NF, D], wbpool)
                if l + 1 < n_layers:
                    load_kc(l + 1)
                    load_vc(l + 1)

                qT = proj_T(wq, bq, xT, qkpool, "qk", i_outer=(l > 0), on_act=True)
                kT = proj_T(wk, bk, xT, qkpool, "qk", i_outer=False, on_act=False)
                vt = vpool.tile([P, NCH, HW_COLS], fsb, tag="v", name=f"vs{l}")
                v_aug_part(vt, wv, bv, xT, list(range(NCH)))

                saT = apool.tile([P, NSUB, S], fsb, tag="attn")
                x_in1 = xipool.tile([P, NSUB, S], fsb, tag="xin")
                sq1 = sqpool.tile([P, NSUB, S], fsb, tag="sq")
                attention(
                    qT, kT, vt, saT, causal_self, smask_sb,
                    mk_post(xT, saT, x_in1, sq1),
                )
                tap(f"sa{l}", saT)
                mean1, s21 = ln_stats(x_in1, sq1)

                # ---- LN1 bubble fillers (encoder-only work) ----
                if l == 0:
                    emit_kc(0)
                    start_vc(0)
                emit_vc_part(l)  # finish this layer's cross V

                g1 = b1g = None
                if not lean_ln:
                    g1 = load_small(ln1g_d, l, [P, NSUB])
                    b1g = load_small(ln1b_d, l, [P, NSUB])
                x1 = ln_finish(x_in1, mean1, s21, g1, b1g, xpool, "x")
                tap(f"x1_{l}", x1)

                # ---- cross attention ----
                wqc = load_w(wq_c_d, l, [P, NSUB, D])
                bqc = load_small(bq_c_d, l, [P, NSUB])
                qcT = proj_T(wqc, bqc, x1, qkpool, "qk", i_outer=True, on_act=True)
                caT = apool.tile([P, NSUB, S], fsb, tag="attn")
                x_in2 = xipool.tile([P, NSUB, S], fsb, tag="xin")
                sq2 = sqpool.tile([P, NSUB, S], fsb, tag="sq")
                attention(
                    qcT, kc_state.pop(l), vc_state[l][0], caT, False, cmask_sb,
                    mk_post(x1, caT, x_in2, sq2),
                )
                vc_state.pop(l)
                tap(f"ca{l}", caT)
                mean2, s22 = ln_stats(x_in2, sq2)

                # ---- LN2 bubble filler: next layer's cross K projection ----
                if l + 1 < n_layers:
                    emit_kc(l + 1)

                g2 = b2g = None
                if not lean_ln:
                    g2 = load_small(ln2g_d, l, [P, NSUB])
                    b2g = load_small(ln2b_d, l, [P, NSUB])
                x2 = ln_finish(x_in2, mean2, s22, g2, b2g, xpool, "x")
                tap(f"x2_{l}", x2)

                # ---- FFN ----
                b1col = load_small(b1_d, l, [P, NF], "b1col")
                ff1 = ffpool.tile([P, NF, S], fsb, tag="ff1")
                # first granule input-subtile-outer (starts at x2 subtile 0)
                pss = [ps_tile() for _ in range(NSUB)]
                for i in range(NSUB):
                    for F in range(NSUB):
                        mm(
                            pss[F][:],
                            w1sb[:, i, F * P : (F + 1) * P],
                            x2[:, i, :],
                            start=(i == 0),
                            stop=(i == NSUB - 1),
                        )
                for F in range(NSUB):
                    nc.vector.tensor_scalar(
                        ff1[:, F, :], pss[F][:], b1col[:, F : F + 1], 0.0,
                        OP.add, OP.max,
                    )
                for F in range(NSUB, NF):
                    ps = ps_tile()
                    for i in range(NSUB):
                        mm(
                            ps[:],
                            w1sb[:, i, F * P : (F + 1) * P],
                            x2[:, i, :],
                            start=(i == 0),
                            stop=(i == NSUB - 1),
                        )
                    nc.vector.tensor_scalar(
                        ff1[:, F, :], ps[:], b1col[:, F : F + 1], 0.0,
                        OP.add, OP.max,
                    )
                if l + 1 < n_layers:
                    load_sa(l + 1)  # prefetch next layer's self-attn weights
                b2col = load_small(b2_d, l, [P, NSUB])
                ffo = apool.tile([P, NSUB, S], fsb, tag="attn")
                x_in3 = xipool.tile([P, NSUB, S], fsb, tag="xin")
                sq3 = sqpool.tile([P, NSUB, S], fsb, tag="sq")
                post3 = mk_post(x2, ffo, x_in3, sq3)
                for j in range(NSUB):
                    ps = ps_tile()
                    for k in range(NF):
                        mm(
                            ps[:],
                            w2sb[:, k, j * P : (j + 1) * P],
                            ff1[:, k, :],
                            start=(k == 0),
                            stop=(k == NF - 1),
                        )
                    nc.vector.tensor_scalar(
                        ffo[:, j, :], ps[:], b2col[:, j : j + 1], None, OP.add
                    )
                    post3(j)
                tap(f"ff{l}", ffo)
                mean3, s23 = ln_stats(x_in3, sq3)

                # ---- LN3 bubble filler: first half of next layer's cross V ----
                if l + 1 < n_layers:
                    start_vc(l + 1)
                    emit_vc_part(l + 1, 2)

                last = l == n_layers - 1
                xT = ln_finish(
                    x_in3, mean3, s23, g2, b2g,
                    xopool if last else xpool, "xo" if last else "x",
                    out_f32=last,
                )

            nc.sync.dma_start(out_d[:], xT[:])

    nc.compile()
    return nc


def _prep_shared(inputs, n_layers):
    """Host-side marshalling of weights into device tile layouts (float32;
    kernel() casts matmul-side arrays to bf16)."""
    g = {}
    emb = np.asarray(inputs["emb"], np.float32)

    def wqk_prep(w):  # [NL, H, D, DK] -> [nl, P, NSUB, D]
        out = np.empty((n_layers, P, NSUB, D), np.float32)
        for l in range(n_layers):
            w2d = np.asarray(w[l], np.float32).transpose(1, 0, 2).reshape(D, H * DK)
            out[l] = w2d.reshape(NSUB, P, H * DK).transpose(1, 0, 2)
        return np.ascontiguousarray(out)

    def wv_prep(w, bv):  # augmented: per head 64 v-cols + ones col
        wout = np.empty((n_layers, P, NSUB, HW_COLS), np.float32)
        brow = np.zeros((n_layers, 1, HW_COLS), np.float32)
        for l in range(n_layers):
            aug = np.zeros((D, HW_COLS), np.float32)
            baug = np.zeros(HW_COLS, np.float32)
            wl = np.asarray(w[l], np.float32)  # [H, D, DVh]
            bl = np.asarray(bv[l], np.float32)  # [H, DVh]
            for h in range(H):
                aug[:, h * 65 : h * 65 + 64] = wl[h]
                baug[h * 65 : h * 65 + 64] = bl[h]
                baug[h * 65 + 64] = 1.0
            wout[l] = aug.reshape(NSUB, P, HW_COLS).transpose(1, 0, 2)
            brow[l, 0] = baug
        return np.ascontiguousarray(wout), brow

    def bcol_prep(b):  # [NL, ...] -> [nl, P, width]
        out = np.stack(
            [_col_layout(np.asarray(b[l], np.float32)) for l in range(n_layers)]
        )
        return np.ascontiguousarray(out)

    g["wq_s"] = wqk_prep(inputs["Wq_s"])
    g["wk_s"] = wqk_prep(inputs["Wk_s"])
    g["wv_s"], g["bv_s"] = wv_prep(inputs["Wv_s"], inputs["bv_s"])
    g["bq_s"] = bcol_prep(inputs["bq_s"])
    g["bk_s"] = bcol_prep(inputs["bk_s"])
    g["wq_c"] = wqk_prep(inputs["Wq_c"])
    g["wk_c"] = wqk_prep(inputs["Wk_c"])
    g["wv_c"], g["bv_c"] = wv_prep(inputs["Wv_c"], inputs["bv_c"])
    g["bq_c"] = bcol_prep(inputs["bq_c"])
    g["bk_c"] = bcol_prep(inputs["bk_c"])

    w1 = np.empty((n_layers, P, NSUB, DFF), np.float32)
    w2 = np.empty((n_layers, P, NF, D), np.float32)
    for l in range(n_layers):
        w1[l] = (
            np.asarray(inputs["W1"][l], np.float32)
            .reshape(NSUB, P, DFF)
            .transpose(1, 0, 2)
        )
        w2[l] = (
            np.asarray(inputs["W2"][l], np.float32)
            .reshape(NF, P, D)
            .transpose(1, 0, 2)
        )
    g["w1"] = np.ascontiguousarray(w1)
    g["w2"] = np.ascontiguousarray(w2)
    g["b1c"] = bcol_prep(inputs["b1"])
    g["b2c"] = bcol_prep(inputs["b2"])
    g["ln1g"] = bcol_prep(inputs["ln1_g"])
    g["ln1b"] = bcol_prep(inputs["ln1_b"])
    g["ln2g"] = bcol_prep(inputs["ln2_g"])
    g["ln2b"] = bcol_prep(inputs["ln2_b"])

    g["ones_row"] = np.ones((1, S), np.float32)
    g["invD_col"] = np.full((P, 1), 1.0 / D, np.float32)
    sel2 = np.zeros((2, P), np.float32)
    sel2[0, 0:64] = 1.0
    sel2[1, 64:128] = 1.0
    g["sel2"] = sel2
    q = np.arange(P)
    g["tri01"] = (q[None, :] >= q[:, None]).astype(np.float32)
    g["ident"] = np.eye(P, dtype=np.float32)
    return g, emb


def _mask_T8(mask_b):
    """[S, S] additive mask -> [P, NCH, S] transposed, pre-scaled by 8."""
    m = np.ascontiguousarray(np.asarray(mask_b, np.float32).T) * 8.0
    return np.ascontiguousarray(m.reshape(NCH, P, S).transpose(1, 0, 2))


# f32 tensors; everything else carries bf16
_F32_KEYS = {
    "bq_s", "bk_s", "bq_c", "bk_c", "b1c", "b2c",
    "ln1g", "ln1b", "ln2g", "ln2b",
}


def kernel(**inputs):
    global LAST_RESULT
    _ensure_path()
    import ml_dtypes
    from concourse.bass_utils import run_bass_kernel_spmd

    n_layers = N_LAYERS
    mm_np = ml_dtypes.bfloat16
    ids = np.asarray(inputs["decoder_input"])
    enc = np.asarray(inputs["encoder_output"], np.float32)
    smask = np.asarray(inputs["self_mask"], np.float32)
    cmask = np.asarray(inputs["cross_mask"], np.float32)

    tril = np.tril(np.ones((S, S), bool))
    canon = np.where(tril, np.float32(0.0), np.float32(-1e9))
    causal_self = all(np.array_equal(smask[b], canon) for b in range(B))
    self_needs_mask = (not causal_self) and bool(np.any(smask != 0.0))
    cross_needs_mask = bool(np.any(cmask != 0.0))
    lean_ln = all(
        np.all(np.asarray(inputs[k], np.float32) == v)
        for k, v in (("ln1_g", 1.0), ("ln2_g", 1.0), ("ln1_b", 0.0), ("ln2_b", 0.0))
    )

    shared, emb = _prep_shared(inputs, n_layers)
    if lean_ln:
        for k in ("ln1g", "ln1b", "ln2g", "ln2b"):
            shared.pop(k)
    shared = {
        k: (v if k in _F32_KEYS else v.astype(mm_np)) for k, v in shared.items()
    }

    key = (n_layers, causal_self, self_needs_mask, cross_needs_mask, lean_ln,
           tuple(TAPS))
    if key not in _BUILD_CACHE:
        _BUILD_CACHE[key] = _build(
            n_layers, causal_self, self_needs_mask, cross_needs_mask, lean_ln,
            tuple(TAPS),
        )
    nc = _BUILD_CACHE[key]

    pe = _pe_table()
    in_maps = []
    for b in range(B):
        m = dict(shared)
        m["x0T"] = _to_T_tiles(emb[ids[b]] + pe).astype(mm_np)
        m["encT"] = _to_T_tiles(enc[b]).astype(mm_np)
        if self_needs_mask:
            m["smaskT8"] = _mask_T8(smask[b]).astype(mm_np)
        if cross_needs_mask:
            m["cmaskT8"] = _mask_T8(cmask[b]).astype(mm_np)
        if not causal_self:
            m.pop("tri01", None)
        if not (self_needs_mask or cross_needs_mask):
            m.pop("ident", None)
        in_maps.append(m)

    res = run_bass_kernel_spmd(nc, in_maps, core_ids=list(range(8)))
    LAST_RESULT = res

    out = np.empty((B, S, D), np.float32)
    for b in range(B):
        xt = np.asarray(res.results[b]["out_xT"], np.float32)  # [P, NSUB, S]
        out[b] = xt.transpose(1, 0, 2).reshape(D, S).T
    return out


# revision 19
# speedup vs baseline: 1.1192x; 1.0314x over previous
"""Trainium2 Bass kernel for a 6-layer transformer decoder (B=8, S=512, D=512,
H=8, DK=DV=64, DFF=2048, vocab 32000).

Strategy: data-parallel over the batch - each of the 8 NeuronCores runs the
full decoder stack for one batch element. No collectives.

v2 (HAM-aware restructure): everything matmul-side runs in bf16 (fp32 PSUM
accumulation). The PE instruction stream is kept dense so the HAM clock gate
stays at 2.4 GHz:
  - attention is software-pipelined across head pairs (scores_{j+1} is queued
    on the PE before head-pair j's denominator broadcast, so the PE never
    waits on the DVE/ACT softmax chain);
  - the encoder-side cross-attention K/V projections are floated into the
    LayerNorm row-chain bubbles (they depend only on the encoder output);
  - LayerNorm stats matmuls are emitted per-subtile as residuals complete,
    and the first projection after each LN accumulates input-subtile-outer
    so it starts as soon as apply writes subtile 0.
Engine balance: exp/softmax + PSUM->SBUF casts on ACT (one activation table:
exp/ln/copy/identity/square - 1/sigma is exp(-0.5*ln(var+eps)), no table
reloads), elementwise + reciprocal + relu on DVE, residuals/squares/causal
masking/half the LN applies on GpSimd (Pool).
"""

import os
import numpy as np

_CONCOURSE_PATHS = ["/opt/trn_rl_repo", "/root/.axon_site/_ro/trn_rl_repo"]


def _ensure_path():
    try:
        import concourse.bass  # noqa: F401
    except Exception:
        import sys

        for p in _CONCOURSE_PATHS:
            if p not in sys.path and os.path.isdir(p):
                sys.path.insert(0, p)


V, D, NL, DK, DVh, H, DFF = 32000, 512, 6, 64, 64, 8, 2048
B, S = 8, 512
EPS = 1e-5
P = 128
NSUB = D // P  # 4 d-subtiles
NCH = S // P  # 4 s-chunks
NF = DFF // P  # 16 dff-chunks
HW_COLS = H * (DVh + 1)  # 520 augmented-v columns

# Debug knobs (test.py may override before calling kernel()).
N_LAYERS = NL
TAPS = ()  # e.g. ("sa0", "x1_0", "ca0", "x2_0", "ff0")
MM_DT = "bf16"  # kept for test.py compat; build is bf16-only

# Results of the last kernel() call (for test.py).
LAST_RESULT = None

_BUILD_CACHE = {}


def _pe_table():
    pos = np.arange(S)[:, None].astype(np.float32)
    i = np.arange(0, D, 2).astype(np.float32)
    ang = pos / np.power(10000.0, i / D)
    pe = np.zeros((S, D), dtype=np.float32)
    pe[:, 0::2] = np.sin(ang)
    pe[:, 1::2] = np.cos(ang)
    return pe


def _to_T_tiles(mat):
    """[S, D]-like -> [P, NSUB, S] transposed-tile layout (mat.T chunked)."""
    t = np.ascontiguousarray(np.asarray(mat, np.float32)).T  # [D, S]
    return np.ascontiguousarray(t.reshape(t.shape[0] // P, P, -1).transpose(1, 0, 2))


def _col_layout(vec):
    """[D]-like -> [P, D//P] per-partition column layout."""
    v = np.asarray(vec, np.float32).reshape(-1)
    return np.ascontiguousarray(v.reshape(v.shape[0] // P, P).T)


def _build(n_layers, causal_self, self_needs_mask, cross_needs_mask, lean_ln, taps):
    _ensure_path()
    import concourse.mybir as mybir
    from concourse import bacc
    from concourse.tile import TileContext

    dt = mybir.dt
    AF = mybir.ActivationFunctionType
    OP = mybir.AluOpType
    f32 = dt.float32
    fsb = dt.bfloat16

    nc = bacc.Bacc("TRN2", target_bir_lowering=False, debug=False, num_devices=8)

    def din(name, shape, d=None):
        return nc.dram_tensor(name, shape, d or fsb, kind="ExternalInput")

    x0T_d = din("x0T", [P, NSUB, S])  # emb rows + positional enc (host)
    encT_d = din("encT", [P, NSUB, S])
    ones_d = din("ones_row", [1, S])
    invD_d = din("invD_col", [P, 1])
    sel2_d = din("sel2", [2, P])
    tri_d = din("tri01", [P, P]) if causal_self else None
    ident_d = din("ident", [P, P]) if (self_needs_mask or cross_needs_mask) else None
    smask_d = din("smaskT8", [P, NCH, S]) if self_needs_mask else None
    cmask_d = din("cmaskT8", [P, NCH, S]) if cross_needs_mask else None

    wq_s_d = din("wq_s", [n_layers, P, NSUB, D])
    wk_s_d = din("wk_s", [n_layers, P, NSUB, D])
    wv_s_d = din("wv_s", [n_layers, P, NSUB, HW_COLS])
    bq_s_d = din("bq_s", [n_layers, P, NSUB], f32)
    bk_s_d = din("bk_s", [n_layers, P, NSUB], f32)
    bv_s_d = din("bv_s", [n_layers, 1, HW_COLS])
    wq_c_d = din("wq_c", [n_layers, P, NSUB, D])
    wk_c_d = din("wk_c", [n_layers, P, NSUB, D])
    wv_c_d = din("wv_c", [n_layers, P, NSUB, HW_COLS])
    bq_c_d = din("bq_c", [n_layers, P, NSUB], f32)
    bk_c_d = din("bk_c", [n_layers, P, NSUB], f32)
    bv_c_d = din("bv_c", [n_layers, 1, HW_COLS])
    w1_d = din("w1", [n_layers, P, NSUB, DFF])
    b1_d = din("b1c", [n_layers, P, NF], f32)
    w2_d = din("w2", [n_layers, P, NF, D])
    b2_d = din("b2c", [n_layers, P, NSUB], f32)
    if not lean_ln:
        ln1g_d = din("ln1g", [n_layers, P, NSUB], f32)
        ln1b_d = din("ln1b", [n_layers, P, NSUB], f32)
        ln2g_d = din("ln2g", [n_layers, P, NSUB], f32)
        ln2b_d = din("ln2b", [n_layers, P, NSUB], f32)

    out_d = nc.dram_tensor("out_xT", [P, NSUB, S], f32, kind="ExternalOutput")
    tap_d = {
        t: nc.dram_tensor(f"tap_{t}", [P, NSUB, S], fsb, kind="ExternalOutput")
        for t in taps
    }

    def mm(out, lhsT, rhs, start, stop):
        nc.tensor.matmul(
            out, lhsT, rhs, start=start, stop=stop, skip_group_check=True
        )

    from contextlib import ExitStack

    with TileContext(nc) as tc:
        with ExitStack() as stack:
            en = stack.enter_context
            en(nc.allow_low_precision(reason="bf16 matmul pipeline"))
            wbpool = en(tc.tile_pool(name="wbig", bufs=2))
            wpool = en(tc.tile_pool(name="wsm", bufs=7))
            spool = en(tc.tile_pool(name="small", bufs=12))
            brpool = en(tc.tile_pool(name="brows", bufs=4))
            qkpool = en(tc.tile_pool(name="qk", bufs=2))
            kcpool = en(tc.tile_pool(name="kc", bufs=2))
            vpool = en(tc.tile_pool(name="v", bufs=2))
            epool = en(tc.tile_pool(name="exp", bufs=4))
            apool = en(tc.tile_pool(name="attn", bufs=2))
            xpool = en(tc.tile_pool(name="x", bufs=2))
            xipool = en(tc.tile_pool(name="xin", bufs=2))
            sqpool = en(tc.tile_pool(name="sq", bufs=2))
            xopool = en(tc.tile_pool(name="xout", bufs=1))
            tpool = en(tc.tile_pool(name="t1", bufs=3))
            rbpool = en(tc.tile_pool(name="rb", bufs=2))
            mbpool = en(tc.tile_pool(name="mbsb", bufs=4))
            ffpool = en(tc.tile_pool(name="ff", bufs=1))
            rpool = en(tc.tile_pool(name="row", bufs=4))
            rcpool = en(tc.tile_pool(name="rcp", bufs=3))
            cpool = en(tc.tile_pool(name="const", bufs=1))
            pspool = en(tc.tile_pool(name="ps", bufs=8, space="PSUM"))
            # ---- constants & persistent activations ----
            ones_sb = cpool.tile([1, S], fsb, tag="c_ones")
            nc.sync.dma_start(ones_sb[:], ones_d[:])
            invD_sb = cpool.tile([P, 1], fsb, tag="c_invD")
            nc.sync.dma_start(invD_sb[:], invD_d[:])
            sel_sb = []
            for u in range(2):
                t = cpool.tile([1, P], fsb, tag=f"c_sel{u}", name=f"sel{u}")
                nc.sync.dma_start(t[:], sel2_d[u : u + 1, :])
                sel_sb.append(t)
            if causal_self:
                tri_sb = cpool.tile([P, P], fsb, tag="c_tri")
                nc.sync.dma_start(tri_sb[:], tri_d[:])
            id_sb = None
            if ident_d is not None:
                id_sb = cpool.tile([P, P], fsb, tag="c_id")
                nc.sync.dma_start(id_sb[:], ident_d[:])
            smask_sb = None
            if self_needs_mask:
                smask_sb = cpool.tile([P, NCH, S], fsb, tag="c_smask")
                nc.sync.dma_start(smask_sb[:], smask_d[:])
            cmask_sb = None
            if cross_needs_mask:
                cmask_sb = cpool.tile([P, NCH, S], fsb, tag="c_cmask")
                nc.sync.dma_start(cmask_sb[:], cmask_d[:])

            eps_sb = cpool.tile([1, 1], f32, tag="c_eps")
            nc.gpsimd.memset(eps_sb[:], float(EPS))

            # Pin the activation table to the one set containing exp AND ln
            # (plus copy/identity/square/relu): without this the compiler
            # ping-pongs tables at every LayerNorm (2 x 1.3us serial, inside
            # the critical row chain).
            act_set_id = 6  # natural_log_exp_and_others in act_info.json
            try:
                from concourse.hw_specs import get_activation_tables

                _names = list(get_activation_tables(nc.m.arch).keys())
                act_set_id = _names.index("natural_log_exp_and_others")
            except Exception:
                pass
            nc.scalar.add_instruction(
                mybir.InstLoadActFuncSet(
                    name=nc.get_next_instruction_name(),
                    ins=[],
                    outs=[],
                    act_func_set_id=act_set_id,
                )
            )

            encT = cpool.tile([P, NSUB, S], fsb, tag="c_enc")
            nc.sync.dma_start(encT[:], encT_d[:])

            xT = xpool.tile([P, NSUB, S], fsb, tag="x")
            nc.sync.dma_start(xT[:], x0T_d[:])

            _psn = [0]

            def ps_tile(n=S, p=P):
                _psn[0] += 1
                return pspool.tile([p, n], f32, tag="ps", name=f"ps{_psn[0]}")

            def load_w(src, l, shape, pool=None):
                t = (pool or wpool).tile(shape, fsb, tag="wt")
                nc.sync.dma_start(t[:], src[l])
                return t

            def load_small(src, l, shape, tag="bcol"):
                if tag == "brow":
                    t = brpool.tile(shape, fsb, tag=tag)
                else:
                    t = spool.tile(shape, f32, tag=tag)
                nc.sync.dma_start(t[:], src[l])
                return t

            def tap(name, tile_):
                if name in tap_d:
                    nc.sync.dma_start(tap_d[name][:], tile_[:])

            def proj_T(w_sb, b_sb, srcT, pool, ptag, i_outer, on_act):
                """dk_all x S projection, transposed output [P, NSUB, S].

                i_outer: accumulate input-subtile-outer so the first matmul
                issues as soon as srcT[:, 0, :] is written."""
                t = pool.tile([P, NSUB, S], fsb, tag=ptag)
                if i_outer:
                    pss = [ps_tile() for _ in range(NSUB)]
                    for i in range(NSUB):
                        for j in range(NSUB):
                            mm(
                                pss[j][:],
                                w_sb[:, i, j * P : (j + 1) * P],
                                srcT[:, i, :],
                                start=(i == 0),
                                stop=(i == NSUB - 1),
                            )
                    for j in range(NSUB):
                        if on_act:
                            nc.scalar.activation(
                                t[:, j, :], pss[j][:], AF.Identity,
                                bias=b_sb[:, j : j + 1],
                            )
                        else:
                            nc.vector.tensor_scalar(
                                t[:, j, :], pss[j][:], b_sb[:, j : j + 1], None,
                                OP.add,
                            )
                else:
                    for j in range(NSUB):
                        ps = ps_tile()
                        for i in range(NSUB):
                            mm(
                                ps[:],
                                w_sb[:, i, j * P : (j + 1) * P],
                                srcT[:, i, :],
                                start=(i == 0),
                                stop=(i == NSUB - 1),
                            )
                        if on_act:
                            nc.scalar.activation(
                                t[:, j, :], ps[:], AF.Identity,
                                bias=b_sb[:, j : j + 1],
                            )
                        else:
                            nc.vector.tensor_scalar(
                                t[:, j, :], ps[:], b_sb[:, j : j + 1], None, OP.add
                            )
                return t

            def v_aug_part(vt, w_sb, brow_sb, srcT, sc_list):
                """augmented v, natural orientation: [P(s), NCH, 520]."""
                half = HW_COLS // 2  # 260
                for sc in sc_list:
                    for hh in range(2):
                        cs, ce = hh * half, (hh + 1) * half
                        ps = ps_tile(n=half)
                        for i in range(NSUB):
                            mm(
                                ps[:],
                                srcT[:, i, sc * P : (sc + 1) * P],
                                w_sb[:, i, cs:ce],
                                start=(i == 0),
                                stop=False,
                            )
                        mm(
                            ps[:],
                            ones_sb[0:1, 0:P],
                            brow_sb[0:1, cs:ce],
                            start=False,
                            stop=True,
                        )
                        nc.vector.tensor_copy(vt[:, sc, cs:ce], ps[:])

            def attention(qT, kT, vt, outT, causal, mask_sb, post_subtile):
                """Software-pipelined attention. Head pair j = heads (2j,2j+1).

                Scores are computed transposed scT[sk, sq] in 128-row sk
                chunks, the two heads of a pair on disjoint PE row groups.
                Softmax denominators come from the augmented-v ones column
                (row 64 of the AV accumulation); 1/denom on DVE, broadcast
                across the pair's 128 output partitions with one K=2 selector
                matmul, one ACT cast to SBUF, multiply on DVE.
                post_subtile(j) emits the follow-on work for subtile j."""
                nj = NSUB
                st = [dict() for _ in range(nj)]

                def s_stage(j):
                    d = st[j]
                    d["exs"] = [
                        epool.tile([P, NCH, S], fsb, tag="exp", name=f"ex{j}_0"),
                        epool.tile([P, NCH, S], fsb, tag="exp", name=f"ex{j}_1"),
                    ]
                    for c in range(NCH):
                        q0 = c * P if causal else 0
                        scs = [ps_tile(), ps_tile()]
                        for u in range(2):
                            ph = u * 64
                            mm(
                                scs[u][:, q0:S],
                                kT[ph : ph + 64, j, c * P : (c + 1) * P],
                                qT[ph : ph + 64, j, q0:S],
                                start=True,
                                stop=(mask_sb is None),
                            )
                            if mask_sb is not None:
                                mm(
                                    scs[u][:, q0:S],
                                    id_sb[:],
                                    mask_sb[:, c, q0:S],
                                    start=False,
                                    stop=True,
                                )
                        for u in range(2):
                            nc.scalar.activation(
                                d["exs"][u][:, c, q0:S], scs[u][:, q0:S], AF.Exp,
                                scale=0.125,
                            )
                            if causal:
                                nc.gpsimd.tensor_tensor(
                                    d["exs"][u][:, c, c * P : (c + 1) * P],
                                    d["exs"][u][:, c, c * P : (c + 1) * P],
                                    tri_sb[:],
                                    OP.mult,
                                )

                def av_stage(j):
                    d = st[j]
                    d["avs"] = [ps_tile(p=65), ps_tile(p=65)]
                    for c in range(NCH):
                        q0 = c * P if causal else 0
                        for u in range(2):
                            h = 2 * j + u
                            mm(
                                d["avs"][u][0:65, q0:S],
                                vt[:, c, h * 65 : (h + 1) * 65],
                                d["exs"][u][:, c, q0:S],
                                start=(c == 0),
                                stop=(c == NCH - 1),
                            )

                def rcpb_stage(j):
                    # denominator rows (bf16, partition 0) -> two K=1 selector
                    # matmuls accumulate the pair broadcast -> one DVE
                    # reciprocal does the PSUM->SBUF move
                    d = st[j]
                    rb2 = ps_tile()
                    for u in range(2):
                        du = rcpool.tile([1, S], fsb, tag="rcp",
                                         name=f"d{j}_{u}")
                        if u == 0:
                            nc.scalar.activation(
                                du[:], d["avs"][u][64:65, :], AF.Copy
                            )
                        else:
                            nc.vector.tensor_copy(du[:], d["avs"][u][64:65, :])
                        mm(
                            rb2[:], sel_sb[u][0:1, :], du[0:1, :],
                            start=(u == 0), stop=(u == 1),
                        )
                    d["rb2sb"] = rbpool.tile([P, S], f32, tag="rb", name=f"rb{j}")
                    nc.vector.reciprocal_approx_fast(d["rb2sb"][:], rb2[:])

                def mult_stage(j):
                    d = st[j]
                    for u in range(2):
                        nc.vector.tensor_tensor(
                            outT[u * 64 : u * 64 + 64, j, :],
                            d["avs"][u][0:64, :],
                            d["rb2sb"][u * 64 : u * 64 + 64, :],
                            OP.mult,
                        )
                    post_subtile(j)

                # emission: PE queue = s0 s1 av0 s2 b0 av1 s3 b1 av2 b2 av3 b3
                s_stage(0)
                s_stage(1)
                av_stage(0)
                for j in range(2, nj):
                    s_stage(j)
                    rcpb_stage(j - 2)
                    av_stage(j - 1)
                    mult_stage(j - 2)
                rcpb_stage(nj - 2)
                av_stage(nj - 1)
                mult_stage(nj - 2)
                rcpb_stage(nj - 1)
                mult_stage(nj - 1)

            def mk_post(base_T, add_T, x_in, sq):
                """residual + square, alternating DVE/Pool per subtile."""

                def post(j):
                    eng = nc.vector if j % 2 == 0 else nc.gpsimd
                    eng.tensor_tensor(
                        x_in[:, j, :], base_T[:, j, :], add_T[:, j, :], OP.add
                    )
                    eng.tensor_tensor(
                        sq[:, j, :], x_in[:, j, :], x_in[:, j, :], OP.mult
                    )

                return post

            def ln_stats(x_in, sq):
                # one PSUM bank: mean row at partition 0, E[x^2] at 32
                stat = ps_tile(p=33)
                for j in range(NSUB):
                    mm(
                        stat[0:1, :],
                        invD_sb[:],
                        x_in[:, j, :],
                        start=(j == 0),
                        stop=(j == NSUB - 1),
                    )
                for j in range(NSUB):
                    mm(
                        stat[32:33, :],
                        invD_sb[:],
                        sq[:, j, :],
                        start=(j == 0),
                        stop=(j == NSUB - 1),
                    )
                return stat[0:1, :], stat[32:33, :]

            def ln_finish(x_in, mean_ps, s2_ps, g_sb, b_sb, out_pool, out_tag,
                          out_f32=False):
                """Row chain + broadcast + apply (DVE/Pool split)."""
                mean_sb = rpool.tile([1, S], fsb, tag="row", name="mean")
                nc.scalar.activation(mean_sb[:], mean_ps[:], AF.Copy)
                msq = rpool.tile([1, S], f32, tag="row", name="msq")
                nc.vector.tensor_tensor(msq[:], mean_sb[:], mean_sb[:], OP.mult)
                var = rpool.tile([1, S], f32, tag="row", name="var")
                nc.vector.tensor_tensor(var[:], s2_ps[:], msq[:], OP.subtract)
                lnv = rpool.tile([1, S], f32, tag="row", name="lnv")
                nc.scalar.activation(lnv[:], var[:], AF.Ln, bias=eps_sb[0:1, 0:1])
                rsd = rpool.tile([1, S], fsb, tag="row", name="rsd")
                nc.scalar.activation(rsd[:], lnv[:], AF.Exp, scale=-0.5)
                mb_ps = ps_tile()
                mm(mb_ps[:], ones_sb[0:1, 0:P], mean_sb[0:1, :], start=True,
                   stop=True)
                sdb_ps = ps_tile()
                mm(sdb_ps[:], ones_sb[0:1, 0:P], rsd[0:1, :], start=True, stop=True)
                mb_sb = mbpool.tile([P, S], fsb, tag="mb", name="mb")
                nc.scalar.activation(mb_sb[:], mb_ps[:], AF.Copy)
                sdb_sb = mbpool.tile([P, S], fsb, tag="mb", name="sdb")
                nc.scalar.activation(sdb_sb[:], sdb_ps[:], AF.Copy)
                xo = out_pool.tile(
                    [P, NSUB, S], f32 if out_f32 else fsb, tag=out_tag
                )
                for i in range(NSUB):
                    eng = nc.vector if i % 2 == 0 else nc.gpsimd
                    t1 = tpool.tile([P, S], fsb, tag="t1", name=f"t1_{i}")
                    eng.tensor_tensor(t1[:], x_in[:, i, :], mb_sb[:], OP.subtract)
                    if lean_ln:
                        eng.tensor_tensor(
                            xo[:, i, :], t1[:], sdb_sb[:], OP.mult
                        )
                    else:
                        t2 = tpool.tile([P, S], fsb, tag="t1", name=f"t2_{i}")
                        eng.tensor_tensor(t2[:], t1[:], sdb_sb[:], OP.mult)
                        eng.tensor_scalar(
                            xo[:, i, :], t2[:], g_sb[:, i : i + 1],
                            b_sb[:, i : i + 1], OP.mult, OP.add,
                        )
                return xo

            # ---- weight prefetch state ----
            kc_w = {}  # l -> (wkc tile, bkc)
            vc_w = {}  # l -> (wvc tile, bvc)
            sa_w = {}  # l -> (wq, wk, wv, bq, bk, bv)
            kc_state = {}  # l -> kcT tile
            vc_state = {}  # l -> [vt tile, w_sb, brow_sb, remaining sc list]

            def load_sa(l):
                sa_w[l] = (
                    load_w(wq_s_d, l, [P, NSUB, D]),
                    load_w(wk_s_d, l, [P, NSUB, D]),
                    load_w(wv_s_d, l, [P, NSUB, HW_COLS]),
                    load_small(bq_s_d, l, [P, NSUB]),
                    load_small(bk_s_d, l, [P, NSUB]),
                    load_small(bv_s_d, l, [1, HW_COLS], "brow"),
                )

            def load_kc(l):
                kc_w[l] = (
                    load_w(wk_c_d, l, [P, NSUB, D]),
                    load_small(bk_c_d, l, [P, NSUB]),
                )

            def load_vc(l):
                vc_w[l] = (
                    load_w(wv_c_d, l, [P, NSUB, HW_COLS]),
                    load_small(bv_c_d, l, [1, HW_COLS], "brow"),
                )

            def emit_kc(l):
                wkc, bkc = kc_w.pop(l)
                kc_state[l] = proj_T(
                    wkc, bkc, encT, kcpool, "kc", i_outer=False, on_act=False
                )

            def start_vc(l):
                wvc, bvc = vc_w.pop(l)
                vt = vpool.tile([P, NCH, HW_COLS], fsb, tag="v", name=f"vc{l}")
                vc_state[l] = [vt, wvc, bvc, list(range(NCH))]

            def emit_vc_part(l, k=None):
                vt, wvc, bvc, rem = vc_state[l]
                k = len(rem) if k is None else k
                scs, vc_state[l][3] = rem[:k], rem[k:]
                v_aug_part(vt, wvc, bvc, encT, scs)

            load_sa(0)
            load_kc(0)
            load_vc(0)
            for l in range(n_layers):
                # ---- self attention ----
                wq, wk, wv, bq, bk, bv = sa_w.pop(l)
                # big FFN weights + next layer's cross weights: DMAs start now
                w1sb = load_w(w1_d, l, [P, NSUB, DFF], wbpool)
                w2sb = load_w(w2_d, l, [P, NF, D], wbpool)
                if l + 1 < n_layers:
                    load_kc(l + 1)
                    load_vc(l + 1)

                qT = proj_T(wq, bq, xT, qkpool, "qk", i_outer=(l > 0), on_act=True)
                kT = proj_T(wk, bk, xT, qkpool, "qk", i_outer=False, on_act=False)
                vt = vpool.tile([P, NCH, HW_COLS], fsb, tag="v", name=f"vs{l}")
                v_aug_part(vt, wv, bv, xT, list(range(NCH)))

                saT = apool.tile([P, NSUB, S], fsb, tag="attn")
                x_in1 = xipool.tile([P, NSUB, S], fsb, tag="xin")
                sq1 = sqpool.tile([P, NSUB, S], fsb, tag="sq")
                attention(
                    qT, kT, vt, saT, causal_self, smask_sb,
                    mk_post(xT, saT, x_in1, sq1),
                )
                tap(f"sa{l}", saT)
                mean1, s21 = ln_stats(x_in1, sq1)

                # ---- LN1 bubble fillers (encoder-only work) ----
                if l == 0:
                    emit_kc(0)
                    start_vc(0)
                emit_vc_part(l)  # finish this layer's cross V

                g1 = b1g = None
                if not lean_ln:
                    g1 = load_small(ln1g_d, l, [P, NSUB])
                    b1g = load_small(ln1b_d, l, [P, NSUB])
                x1 = ln_finish(x_in1, mean1, s21, g1, b1g, xpool, "x")
                tap(f"x1_{l}", x1)

                # ---- cross attention ----
                wqc = load_w(wq_c_d, l, [P, NSUB, D])
                bqc = load_small(bq_c_d, l, [P, NSUB])
                qcT = proj_T(wqc, bqc, x1, qkpool, "qk", i_outer=True, on_act=True)
                caT = apool.tile([P, NSUB, S], fsb, tag="attn")
                x_in2 = xipool.tile([P, NSUB, S], fsb, tag="xin")
                sq2 = sqpool.tile([P, NSUB, S], fsb, tag="sq")
                attention(
                    qcT, kc_state.pop(l), vc_state[l][0], caT, False, cmask_sb,
                    mk_post(x1, caT, x_in2, sq2),
                )
                vc_state.pop(l)
                tap(f"ca{l}", caT)
                mean2, s22 = ln_stats(x_in2, sq2)

                # ---- LN2 bubble filler: next layer's cross K projection ----
                if l + 1 < n_layers:
                    emit_kc(l + 1)

                g2 = b2g = None
                if not lean_ln:
                    g2 = load_small(ln2g_d, l, [P, NSUB])
                    b2g = load_small(ln2b_d, l, [P, NSUB])
                x2 = ln_finish(x_in2, mean2, s22, g2, b2g, xpool, "x")
                tap(f"x2_{l}", x2)

                # ---- FFN ----
                b1col = load_small(b1_d, l, [P, NF], "b1col")
                ff1 = ffpool.tile([P, NF, S], fsb, tag="ff1")
                # first granule input-subtile-outer (starts at x2 subtile 0)
                pss = [ps_tile() for _ in range(NSUB)]
                for i in range(NSUB):
                    for F in range(NSUB):
                        mm(
                            pss[F][:],
                            w1sb[:, i, F * P : (F + 1) * P],
                            x2[:, i, :],
                            start=(i == 0),
                            stop=(i == NSUB - 1),
                        )
                for F in range(NSUB):
                    nc.vector.tensor_scalar(
                        ff1[:, F, :], pss[F][:], b1col[:, F : F + 1], 0.0,
                        OP.add, OP.max,
                    )
                for F in range(NSUB, NF):
                    ps = ps_tile()
                    for i in range(NSUB):
                        mm(
                            ps[:],
                            w1sb[:, i, F * P : (F + 1) * P],
                            x2[:, i, :],
                            start=(i == 0),
                            stop=(i == NSUB - 1),
                        )
                    nc.vector.tensor_scalar(
                        ff1[:, F, :], ps[:], b1col[:, F : F + 1], 0.0,
                        OP.add, OP.max,
                    )
                if l + 1 < n_layers:
                    load_sa(l + 1)  # prefetch next layer's self-attn weights
                b2col = load_small(b2_d, l, [P, NSUB])
                ffo = apool.tile([P, NSUB, S], fsb, tag="attn")
                x_in3 = xipool.tile([P, NSUB, S], fsb, tag="xin")
                sq3 = sqpool.tile([P, NSUB, S], fsb, tag="sq")
                post3 = mk_post(x2, ffo, x_in3, sq3)
                for j in range(NSUB):
                    ps = ps_tile()
                    for k in range(NF):
                        mm(
                            ps[:],
                            w2sb[:, k, j * P : (j + 1) * P],
                            ff1[:, k, :],
                            start=(k == 0),
                            stop=(k == NF - 1),
                        )
                    nc.vector.tensor_scalar(
                        ffo[:, j, :], ps[:], b2col[:, j : j + 1], None, OP.add
                    )
                    post3(j)
                tap(f"ff{l}", ffo)
                mean3, s23 = ln_stats(x_in3, sq3)

                # ---- LN3 bubble filler: first half of next layer's cross V ----
                if l + 1 < n_layers:
                    start_vc(l + 1)
                    emit_vc_part(l + 1, 2)

                last = l == n_layers - 1
                xT = ln_finish(
                    x_in3, mean3, s23, g2, b2g,
                    xopool if last else xpool, "xo" if last else "x",
                    out_f32=last,
                )

            nc.sync.dma_start(out_d[:], xT[:])

    nc.compile()
    return nc


def _prep_shared(inputs, n_layers):
    """Host-side marshalling of weights into device tile layouts (float32;
    kernel() casts matmul-side arrays to bf16)."""
    g = {}
    emb = np.asarray(inputs["emb"], np.float32)

    def wqk_prep(w):  # [NL, H, D, DK] -> [nl, P, NSUB, D]
        out = np.empty((n_layers, P, NSUB, D), np.float32)
        for l in range(n_layers):
            w2d = np.asarray(w[l], np.float32).transpose(1, 0, 2).reshape(D, H * DK)
            out[l] = w2d.reshape(NSUB, P, H * DK).transpose(1, 0, 2)
        return np.ascontiguousarray(out)

    def wv_prep(w, bv):  # augmented: per head 64 v-cols + ones col
        wout = np.empty((n_layers, P, NSUB, HW_COLS), np.float32)
        brow = np.zeros((n_layers, 1, HW_COLS), np.float32)
        for l in range(n_layers):
            aug = np.zeros((D, HW_COLS), np.float32)
            baug = np.zeros(HW_COLS, np.float32)
            wl = np.asarray(w[l], np.float32)  # [H, D, DVh]
            bl = np.asarray(bv[l], np.float32)  # [H, DVh]
            for h in range(H):
                aug[:, h * 65 : h * 65 + 64] = wl[h]
                baug[h * 65 : h * 65 + 64] = bl[h]
                baug[h * 65 + 64] = 1.0
            wout[l] = aug.reshape(NSUB, P, HW_COLS).transpose(1, 0, 2)
            brow[l, 0] = baug
        return np.ascontiguousarray(wout), brow

    def bcol_prep(b):  # [NL, ...] -> [nl, P, width]
        out = np.stack(
            [_col_layout(np.asarray(b[l], np.float32)) for l in range(n_layers)]
        )
        return np.ascontiguousarray(out)

    g["wq_s"] = wqk_prep(inputs["Wq_s"])
    g["wk_s"] = wqk_prep(inputs["Wk_s"])
    g["wv_s"], g["bv_s"] = wv_prep(inputs["Wv_s"], inputs["bv_s"])
    g["bq_s"] = bcol_prep(inputs["bq_s"])
    g["bk_s"] = bcol_prep(inputs["bk_s"])
    g["wq_c"] = wqk_prep(inputs["Wq_c"])
    g["wk_c"] = wqk_prep(inputs["Wk_c"])
    g["wv_c"], g["bv_c"] = wv_prep(inputs["Wv_c"], inputs["bv_c"])
    g["bq_c"] = bcol_prep(inputs["bq_c"])
    g["bk_c"] = bcol_prep(inputs["bk_c"])

    w1 = np.empty((n_layers, P, NSUB, DFF), np.float32)
    w2 = np.empty((n_layers, P, NF, D), np.float32)
    for l in range(n_layers):
        w1[l] = (
            np.asarray(inputs["W1"][l], np.float32)
            .reshape(NSUB, P, DFF)
            .transpose(1, 0, 2)
        )
        w2[l] = (
            np.asarray(inputs["W2"][l], np.float32)
            .reshape(NF, P, D)
            .transpose(1, 0, 2)
        )
    g["w1"] = np.ascontiguousarray(w1)
    g["w2"] = np.ascontiguousarray(w2)
    g["b1c"] = bcol_prep(inputs["b1"])
    g["b2c"] = bcol_prep(inputs["b2"])
    g["ln1g"] = bcol_prep(inputs["ln1_g"])
    g["ln1b"] = bcol_prep(inputs["ln1_b"])
    g["ln2g"] = bcol_prep(inputs["ln2_g"])
    g["ln2b"] = bcol_prep(inputs["ln2_b"])

    g["ones_row"] = np.ones((1, S), np.float32)
    g["invD_col"] = np.full((P, 1), 1.0 / D, np.float32)
    sel2 = np.zeros((2, P), np.float32)
    sel2[0, 0:64] = 1.0
    sel2[1, 64:128] = 1.0
    g["sel2"] = sel2
    q = np.arange(P)
    g["tri01"] = (q[None, :] >= q[:, None]).astype(np.float32)
    g["ident"] = np.eye(P, dtype=np.float32)
    return g, emb


def _mask_T8(mask_b):
    """[S, S] additive mask -> [P, NCH, S] transposed, pre-scaled by 8."""
    m = np.ascontiguousarray(np.asarray(mask_b, np.float32).T) * 8.0
    return np.ascontiguousarray(m.reshape(NCH, P, S).transpose(1, 0, 2))


# f32 tensors; everything else carries bf16
_F32_KEYS = {
    "bq_s", "bk_s", "bq_c", "bk_c", "b1c", "b2c",
    "ln1g", "ln1b", "ln2g", "ln2b",
}


def kernel(**inputs):
    global LAST_RESULT
    _ensure_path()
    import ml_dtypes
    from concourse.bass_utils import run_bass_kernel_spmd

    n_layers = N_LAYERS
    mm_np = ml_dtypes.bfloat16
    ids = np.asarray(inputs["decoder_input"])
    enc = np.asarray(inputs["encoder_output"], np.float32)
    smask = np.asarray(inputs["self_mask"], np.float32)
    cmask = np.asarray(inputs["cross_mask"], np.float32)

    tril = np.tril(np.ones((S, S), bool))
    canon = np.where(tril, np.float32(0.0), np.float32(-1e9))
    causal_self = all(np.array_equal(smask[b], canon) for b in range(B))
    self_needs_mask = (not causal_self) and bool(np.any(smask != 0.0))
    cross_needs_mask = bool(np.any(cmask != 0.0))
    lean_ln = all(
        np.all(np.asarray(inputs[k], np.float32) == v)
        for k, v in (("ln1_g", 1.0), ("ln2_g", 1.0), ("ln1_b", 0.0), ("ln2_b", 0.0))
    )

    shared, emb = _prep_shared(inputs, n_layers)
    if lean_ln:
        for k in ("ln1g", "ln1b", "ln2g", "ln2b"):
            shared.pop(k)
    shared = {
        k: (v if k in _F32_KEYS else v.astype(mm_np)) for k, v in shared.items()
    }

    key = (n_layers, causal_self, self_needs_mask, cross_needs_mask, lean_ln,
           tuple(TAPS))
    if key not in _BUILD_CACHE:
        _BUILD_CACHE[key] = _build(
            n_layers, causal_self, self_needs_mask, cross_needs_mask, lean_ln,
            tuple(TAPS),
        )
    nc = _BUILD_CACHE[key]

    pe = _pe_table()
    in_maps = []
    for b in range(B):
        m = dict(shared)
        m["x0T"] = _to_T_tiles(emb[ids[b]] + pe).astype(mm_np)
        m["encT"] = _to_T_tiles(enc[b]).astype(mm_np)
        if self_needs_mask:
            m["smaskT8"] = _mask_T8(smask[b]).astype(mm_np)
        if cross_needs_mask:
            m["cmaskT8"] = _mask_T8(cmask[b]).astype(mm_np)
        if not causal_self:
            m.pop("tri01", None)
        if not (self_needs_mask or cross_needs_mask):
            m.pop("ident", None)
        in_maps.append(m)

    res = run_bass_kernel_spmd(nc, in_maps, core_ids=list(range(8)))
    LAST_RESULT = res

    out = np.empty((B, S, D), np.float32)
    for b in range(B):
        xt = np.asarray(res.results[b]["out_xT"], np.float32)  # [P, NSUB, S]
        out[b] = xt.transpose(1, 0, 2).reshape(D, S).T
    return out
